# revision 1
# baseline (speedup 1.0000x reference)
"""BiRGAT (bipartite 2-layer GATv2) Trainium2 kernel, 8-core SPMD.

Strategy: destination-tile sharding. Gene dsts padded to 160 tiles of 128
(20 tiles/core), sample dsts 32 tiles (4/core), dealt to cores by sorted
chunk count so every core runs an identical baked per-slot chunk schedule.
Edge-phase per 128-edge chunk: indirect-DMA gather of source rows, GATv2
attention (Prelu + fused dot via scalar_tensor_tensor accum_out, exp),
one-hot matmul scatter-add of [messages | ea] into PSUM. Tile evacuation
does softmax normalization (no segment_max: alpha std ~0.3; the reference's
den+1e-16 makes max-subtraction irrelevant), bias, ELU, residuals.
Source-side tables are AllGathered between phases (overlapped with edge
compute by the Tile scheduler).
"""
import sys

sys.path.insert(0, "/opt/trn_rl_repo")

import numpy as np
from contextlib import ExitStack

import concourse.bass as bass
import concourse.tile as tile
from concourse import bacc, mybir
from concourse.bass_utils import run_bass_kernel_spmd
from concourse.masks import make_identity

P = 128
NCORES = 8
NS, NG, E = 4096, 20000, 131072
DIN, H, C1, C3 = 256, 4, 64, 128
HC1, HC3 = H * C1, H * C3          # 256, 512
NGP = 20480                        # genes padded to 160 tiles
NST, NGT = NS // P, NGP // P       # 32, 160
S_PER_CORE, G_PER_CORE = NST // NCORES, NGT // NCORES   # 4, 20
SROWS, GROWS = S_PER_CORE * P, G_PER_CORE * P           # 512, 2560

F32 = mybir.dt.float32
I32 = mybir.dt.int32
AF = mybir.ActivationFunctionType
OP = mybir.AluOpType

PAD_LOC = 200.0   # dst-local sentinel for padded edges (never equals 0..127)


# ---------------------------------------------------------------- host plan

def _deal_tiles(dst, n_tiles, per_core):
    """Deal dst tiles to cores by sorted chunk count. Returns
    assign[slot, core] -> tile id, sched[slot] -> chunks, per-tile counts."""
    tcnt = np.bincount(dst // P, minlength=n_tiles)
    chunks = np.maximum((tcnt + P - 1) // P, 1)
    order = np.argsort(-chunks, kind="stable")
    assign = order.reshape(per_core, NCORES)
    sched = chunks[assign].max(axis=1)
    return assign, sched.astype(int), tcnt


def _edge_arrays(src, dst, assign, sched, src_row_map, core):
    """Per-core edge chunk arrays for one relation.
    Returns src_rows [P, NCH] i32, dstrow [P, NCH] i32, dstloc [P, NCH] f32."""
    nch = int(sched.sum())
    src_rows = np.zeros((nch, P), np.int32)
    dstrow = np.zeros((nch, P), np.int32)
    dstloc = np.full((nch, P), PAD_LOC, np.float32)
    tile_of = dst // P
    ci = 0
    for slot in range(len(sched)):
        t = assign[slot, core]
        e = np.nonzero(tile_of == t)[0]
        n = len(e)
        want = sched[slot] * P
        s = np.zeros(want, np.int32)
        dl = np.full(want, PAD_LOC, np.float32)
        dr = np.zeros(want, np.int32)
        s[:n] = src_row_map[src[e]]
        dl[:n] = (dst[e] % P).astype(np.float32)
        dr[:n] = slot * P + dst[e] % P
        src_rows[ci:ci + sched[slot]] = s.reshape(-1, P)
        dstloc[ci:ci + sched[slot]] = dl.reshape(-1, P)
        dstrow[ci:ci + sched[slot]] = dr.reshape(-1, P)
        ci += sched[slot]
    return src_rows.T.copy(), dstrow.T.copy(), dstloc.T.copy()


def _bcast(v, p=P):
    return np.broadcast_to(np.asarray(v, np.float32).reshape(1, -1),
                           (p, len(np.asarray(v).reshape(-1)))).copy()


def _plan(inputs):
    sg_src = np.asarray(inputs["sg_src"]); sg_dst = np.asarray(inputs["sg_dst"])
    gs_src = np.asarray(inputs["gs_src"]); gs_dst = np.asarray(inputs["gs_dst"])

    g_assign, g_sched, _ = _deal_tiles(sg_dst, NGT, G_PER_CORE)
    s_assign, s_sched, _ = _deal_tiles(gs_dst, NST, S_PER_CORE)

    # tile -> (owner core, slot)
    g_owner = np.zeros(NGT, np.int32); g_slot = np.zeros(NGT, np.int32)
    for slot in range(G_PER_CORE):
        for c in range(NCORES):
            g_owner[g_assign[slot, c]] = c
            g_slot[g_assign[slot, c]] = slot
    s_owner = np.zeros(NST, np.int32); s_slot = np.zeros(NST, np.int32)
    for slot in range(S_PER_CORE):
        for c in range(NCORES):
            s_owner[s_assign[slot, c]] = c
            s_slot[s_assign[slot, c]] = slot

    sid = np.arange(NS)
    srow_tbl = s_owner[sid // P] * SROWS + s_slot[sid // P] * P + sid % P
    gid = np.arange(NG)
    grow_tbl = g_owner[gid // P] * GROWS + g_slot[gid // P] * P + gid % P

    plan = {
        "g_assign": g_assign, "g_sched": g_sched,
        "s_assign": s_assign, "s_sched": s_sched,
        "srow_tbl": srow_tbl, "grow_tbl": grow_tbl,
    }

    x_sample = np.asarray(inputs["x_sample"], np.float32)
    x_gene = np.asarray(inputs["x_gene"], np.float32)

    in_maps = []
    for c in range(NCORES):
        # node rows owned by this core, in slot order
        s_tiles = s_assign[:, c]
        xs_own = x_sample.reshape(NST, P, DIN)[s_tiles].reshape(SROWS, DIN)
        g_tiles = g_assign[:, c]
        xg_own = np.zeros((GROWS, DIN), np.float32)
        for i, t in enumerate(g_tiles):
            lo = t * P
            if lo < NG:
                n = min(P, NG - lo)
                xg_own[i * P:i * P + n] = x_gene[lo:lo + n]

        sgS, sgR, sgL = _edge_arrays(sg_src, sg_dst, g_assign, g_sched,
                                     srow_tbl, c)
        gsS, gsR, gsL = _edge_arrays(gs_src, gs_dst, s_assign, s_sched,
                                     grow_tbl, c)
        gsS3 = _edge_arrays(gs_src, gs_dst, s_assign, s_sched,
                            grow_tbl, c)[0]  # same rows; tbl3 shares layout

        m = {
            "xs_own": np.ascontiguousarray(xs_own),
            "xg_own": np.ascontiguousarray(xg_own),
            "Wl1_sg": np.asarray(inputs["Wl1_sg"], np.float32),
            "Wr1_sg": np.asarray(inputs["Wr1_sg"], np.float32),
            "Wl1_gs": np.asarray(inputs["Wl1_gs"], np.float32),
            "Wr1_gs": np.asarray(inputs["Wr1_gs"], np.float32),
            "Wl3": np.asarray(inputs["Wl3_gs"], np.float32),
            "Wr3": np.asarray(inputs["Wr3_gs"], np.float32),
            "sl1_W": np.asarray(inputs["sl1_W"], np.float32),
            "sl3_W": np.asarray(inputs["sl3_W"], np.float32),
            "att1_sg_b": _bcast(np.asarray(inputs["att1_sg"]).reshape(-1)),
            "att1_gs_b": _bcast(np.asarray(inputs["att1_gs"]).reshape(-1)),
            "att3_b": _bcast(np.asarray(inputs["att3_gs"]).reshape(-1)),
            "bl1_sg_b": _bcast(inputs["bl1_sg"]),
            "br1_sg_b": _bcast(inputs["br1_sg"]),
            "bl1_gs_b": _bcast(inputs["bl1_gs"]),
            "br1_gs_b": _bcast(inputs["br1_gs"]),
            "bias1_sg_b": _bcast(inputs["bias1_sg"]),
            "bias1_gs_b": _bcast(inputs["bias1_gs"]),
            "bl3_b": _bcast(inputs["bl3_gs"]),
            "br3_b": _bcast(inputs["br3_gs"]),
            "bias3_b": _bcast(inputs["bias3_gs"]),
            "sl1_b_b": _bcast(inputs["sl1_b"]),
            "sl3_b_b": _bcast(inputs["sl3_b"]),
            "sg_srcr": sgS, "sg_dstl": sgL,
            "gs_srcr": gsS, "gs_dstl": gsL,
            "gs_srcr3": gsS3,
        }
        in_maps.append(m)
    return plan, in_maps


# ------------------------------------------------------------- device build

def _load_w(nc, pool, w_dram, kdim, n, tag):
    """Load [kdim, n] weight into SBUF as [128, kdim//128, n] rhs tiles."""
    kc = kdim // P
    t = pool.tile([P, kc, n], F32, tag=tag)
    nc.sync.dma_start(t[:], w_dram[:].rearrange("(c p) n -> p c n", p=P))
    return t


def _transpose2(nc, sb, psp, ident, x_ap, kc):
    """PE-transpose x [128, kc*128] -> list of kc SBUF tiles [128,128]."""
    outs = []
    for k in range(kc):
        pt = psp.tile([P, P], F32, space="PSUM", tag="transp")
        nc.tensor.transpose(out=pt[:], in_=x_ap[:, k * P:(k + 1) * P],
                            identity=ident[:])
        st = sb.tile([P, P], F32, tag="transs")
        nc.scalar.copy(st[:], pt[:])
        outs.append(st)
    return outs


F32R = mybir.dt.float32r


def _r(ap):
    """Matmul operand passthrough (fp32r rejected by walrus: producers
    must pre-round; revisit if PE becomes the bottleneck)."""
    return ap


def _ap3(base_ap, h, c, mid, inner):
    """[128, h, c] view over base_ap's tensor with given free strides."""
    return bass.AP(base_ap.tensor, base_ap.offset,
                   [[base_ap.ap[0][0], P], [mid, h], [inner, c]])


def _mm_kc(nc, psum_ap, xT, w_sb, n):
    kc = len(xT)
    for k in range(kc):
        nc.tensor.matmul(psum_ap, lhsT=_r(xT[k][:]), rhs=_r(w_sb[:, k, :n]),
                         start=(k == 0), stop=(k == kc - 1))


def _elu(nc, sb, out_ap, y_ap, w):
    """out = elu(y) = (relu(y) - 1) + exp(min(y, 0)); [128, w] tiles."""
    m = sb.tile([P, w], F32, tag="elu_m")
    nc.vector.tensor_scalar(out=m[:], in0=y_ap, scalar1=0.0, scalar2=None,
                            op0=OP.min)
    e = sb.tile([P, w], F32, tag="elu_e")
    nc.scalar.activation(e[:], m[:], AF.Exp)
    r = sb.tile([P, w], F32, tag="elu_r")
    nc.scalar.activation(r[:], y_ap, AF.Relu)
    nc.vector.scalar_tensor_tensor(out=out_ap, in0=r[:], scalar=-1.0,
                                   in1=e[:], op0=OP.add, op1=OP.add)


def _build(g_sched, s_sched):
    nsg = int(g_sched.sum())
    ngs = int(s_sched.sum())
    nc = bacc.Bacc("TRN2", target_bir_lowering=False, debug=False,
                   num_devices=NCORES)

    ei = lambda name, shape, dt=F32: nc.dram_tensor(name, shape, dt,
                                                    kind="ExternalInput")
    xs_own = ei("xs_own", [SROWS, DIN]); xg_own = ei("xg_own", [GROWS, DIN])
    Wl1_sg = ei("Wl1_sg", [DIN, HC1]); Wr1_sg = ei("Wr1_sg", [DIN, HC1])
    Wl1_gs = ei("Wl1_gs", [DIN, HC1]); Wr1_gs = ei("Wr1_gs", [DIN, HC1])
    Wl3 = ei("Wl3", [HC1, HC3]); Wr3 = ei("Wr3", [HC1, HC3])
    sl1_W = ei("sl1_W", [DIN, C1]); sl3_W = ei("sl3_W", [HC1, C3])
    att1_sg_b = ei("att1_sg_b", [P, HC1]); att1_gs_b = ei("att1_gs_b", [P, HC1])
    att3_b = ei("att3_b", [P, HC3])
    bl1_sg_b = ei("bl1_sg_b", [P, HC1]); br1_sg_b = ei("br1_sg_b", [P, HC1])
    bl1_gs_b = ei("bl1_gs_b", [P, HC1]); br1_gs_b = ei("br1_gs_b", [P, HC1])
    bias1_sg_b = ei("bias1_sg_b", [P, HC1]); bias1_gs_b = ei("bias1_gs_b", [P, HC1])
    bl3_b = ei("bl3_b", [P, HC3]); br3_b = ei("br3_b", [P, HC3])
    bias3_b = ei("bias3_b", [P, C3])
    sl1_b_b = ei("sl1_b_b", [P, C1]); sl3_b_b = ei("sl3_b_b", [P, C3])
    sg_srcr = ei("sg_srcr", [P, nsg], I32)
    sg_dstl = ei("sg_dstl", [P, nsg]); gs_srcr = ei("gs_srcr", [P, ngs], I32)
    gs_dstl = ei("gs_dstl", [P, ngs])
    gs_srcr3 = ei("gs_srcr3", [P, ngs], I32)

    out_own = nc.dram_tensor("out_own", [SROWS, C3], F32, kind="ExternalOutput")

    # DRAM scratch
    agin_s = nc.dram_tensor("agin_s", [SROWS, HC1], F32R)
    agin_g = nc.dram_tensor("agin_g", [GROWS, HC1], F32R)
    agin_3 = nc.dram_tensor("agin_3", [GROWS, HC3], F32R)
    tbl_s = nc.dram_tensor("tbl_s", [NS, HC1], F32R, addr_space="Shared")
    tbl_g = nc.dram_tensor("tbl_g", [NGP, HC1], F32R, addr_space="Shared")
    tbl_3 = nc.dram_tensor("tbl_3", [NGP, HC3], F32R, addr_space="Shared")
    xr1_sg = nc.dram_tensor("xr1_sg", [GROWS, HC1], F32)
    xr1_gs = nc.dram_tensor("xr1_gs", [SROWS, HC1], F32)
    xr3 = nc.dram_tensor("xr3", [SROWS, HC3], F32)

    RG = [list(range(NCORES))]

    with tile.TileContext(nc) as tc, ExitStack() as ctx:
        res = ctx.enter_context(tc.tile_pool(name="res", bufs=1))
        wp = ctx.enter_context(tc.tile_pool(name="wp", bufs=1))
        sb = ctx.enter_context(tc.tile_pool(name="sb", bufs=6))
        ev = ctx.enter_context(tc.tile_pool(name="ev", bufs=2))
        psp = ctx.enter_context(tc.tile_pool(name="psp", bufs=2, space="PSUM"))
        ps1 = ctx.enter_context(tc.tile_pool(name="ps1", bufs=1, space="PSUM"))
        pse = ctx.enter_context(tc.tile_pool(name="pse", bufs=1, space="PSUM"))
        psx = ctx.enter_context(tc.tile_pool(name="psx", bufs=4, space="PSUM"))

        ident = res.tile([P, P], F32)
        make_identity(nc, ident[:])
        ident_r = res.tile([P, P], F32R)
        nc.scalar.copy(ident_r[:], ident[:])
        iota = res.tile([P, P], F32)
        nc.gpsimd.iota(iota[:], pattern=[[1, P]], base=0, channel_multiplier=0,
                       allow_small_or_imprecise_dtypes=True)

        def rload(name, dram, shape, dt=F32):
            t = res.tile(shape, dt, tag=name)
            nc.sync.dma_start(t[:], dram[:])
            return t

        att1_sg_t = rload("a1s", att1_sg_b, [P, HC1])
        att1_gs_t = rload("a1g", att1_gs_b, [P, HC1])
        att3_t = rload("a3", att3_b, [P, HC3])
        bias1_sg_t = rload("b1s", bias1_sg_b, [P, HC1])
        bias1_gs_t = rload("b1g", bias1_gs_b, [P, HC1])
        bias3_t = rload("b3", bias3_b, [P, C3])
        sg_srcr_t = rload("sgs", sg_srcr, [P, nsg], I32)
        sg_dstl_t = rload("sgl", sg_dstl, [P, nsg])
        gs_srcr_t = rload("gss", gs_srcr, [P, ngs], I32)
        gs_dstl_t = rload("gsl", gs_dstl, [P, ngs])
        gs_srcr3_t = rload("gs3", gs_srcr3, [P, ngs], I32)

        sl1_sb = res.tile([P, S_PER_CORE * C1], F32)   # sl1 rows per slot
        sl3_sb = res.tile([P, S_PER_CORE * C3], F32)   # sl3 rows per slot

        # weights (rhs layout [128, kc, n])
        Wl1_sg_t = _load_w(nc, wp, Wl1_sg, DIN, HC1, "Wl1_sg")
        Wr1_sg_t = _load_w(nc, wp, Wr1_sg, DIN, HC1, "Wr1_sg")
        Wl1_gs_t = _load_w(nc, wp, Wl1_gs, DIN, HC1, "Wl1_gs")
        Wr1_gs_t = _load_w(nc, wp, Wr1_gs, DIN, HC1, "Wr1_gs")
        Wl3_t = _load_w(nc, wp, Wl3, HC1, HC3, "Wl3")
        Wr3_t = _load_w(nc, wp, Wr3, HC1, HC3, "Wr3")
        sl1_W_t = _load_w(nc, wp, sl1_W, DIN, C1, "sl1_W")
        sl3_W_t = _load_w(nc, wp, sl3_W, HC1, C3, "sl3_W")
        bl1_sg_t = rload("bl1s", bl1_sg_b, [P, HC1])
        br1_sg_t = rload("br1s", br1_sg_b, [P, HC1])
        bl1_gs_t = rload("bl1g", bl1_gs_b, [P, HC1])
        br1_gs_t = rload("br1g", br1_gs_b, [P, HC1])
        bl3_t = rload("bl3", bl3_b, [P, HC3])
        br3_t = rload("br3", br3_b, [P, HC3])
        sl1_b_t = rload("sl1b", sl1_b_b, [P, C1])
        sl3_b_t = rload("sl3b", sl3_b_b, [P, C3])

        def dense_out(xT, w_sb, n, bias_t, dst_ap=None, sbuf_dst=None,
                      rdt=F32):
            pt = ps1.tile([P, n], F32, space="PSUM", tag="aux")
            _mm_kc(nc, pt[:], xT, w_sb, n)
            o = sbuf_dst if sbuf_dst is not None else sb.tile([P, n], rdt,
                                                             tag="dout")
            nc.vector.tensor_tensor(out=o[:] if sbuf_dst is None else sbuf_dst,
                                    in0=pt[:, :n], in1=bias_t[:, :n],
                                    op=OP.add)
            if dst_ap is not None:
                nc.sync.dma_start(dst_ap, o[:])
            return o

        # ---- phase A: sample node tables, AG1a
        for i in range(S_PER_CORE):
            xs = sb.tile([P, DIN], F32, tag="xnode")
            nc.sync.dma_start(xs[:], xs_own[i * P:(i + 1) * P, :])
            xT = _transpose2(nc, sb, psp, ident, xs[:], DIN // P)
            dense_out(xT, Wl1_sg_t, HC1, bl1_sg_t,
                      dst_ap=agin_s[i * P:(i + 1) * P, :], rdt=F32R)
            dense_out(xT, Wr1_gs_t, HC1, br1_gs_t,
                      dst_ap=xr1_gs[i * P:(i + 1) * P, :])
            dense_out(xT, sl1_W_t, C1, sl1_b_t,
                      sbuf_dst=sl1_sb[:, i * C1:(i + 1) * C1])
        nc.gpsimd.collective_compute("AllGather", OP.bypass, replica_groups=RG,
                                     ins=[agin_s[:]], outs=[tbl_s[:]])

        # ---- phase A: gene node tables, AG1b
        for j in range(G_PER_CORE):
            xg = sb.tile([P, DIN], F32, tag="xnode")
            nc.sync.dma_start(xg[:], xg_own[j * P:(j + 1) * P, :])
            xT = _transpose2(nc, sb, psp, ident, xg[:], DIN // P)
            dense_out(xT, Wl1_gs_t, HC1, bl1_gs_t,
                      dst_ap=agin_g[j * P:(j + 1) * P, :], rdt=F32R)
            dense_out(xT, Wr1_sg_t, HC1, br1_sg_t,
                      dst_ap=xr1_sg[j * P:(j + 1) * P, :])
        nc.gpsimd.collective_compute("AllGather", OP.bypass, replica_groups=RG,
                                     ins=[agin_g[:]], outs=[tbl_g[:]])

        # ---- edge chunk body
        def edge_chunk(ci, srcr_t, dstl_t, tbl, xr_slot, att_t,
                       psum_m, psum_d, first, last, w):
            xl = sb.tile([P, w], F32R, tag=f"xl{w}")
            nc.gpsimd.indirect_dma_start(
                out=xl[:], out_offset=None, in_=tbl[:],
                in_offset=bass.IndirectOffsetOnAxis(ap=srcr_t[:, ci:ci + 1],
                                                    axis=0))
            # one-hot of dst-local index (also kills padded edges)
            onehot = sb.tile([P, P], F32R, tag="onehot")
            nc.vector.tensor_scalar(out=onehot[:], in0=iota[:],
                                    scalar1=dstl_t[:, ci:ci + 1], scalar2=None,
                                    op0=OP.is_equal)
            # xr[e,:] = xr_slot[dstloc_e,:] via ohT.T @ xr_slot on PE
            ohTp = psp.tile([P, P], F32R, space="PSUM", tag="transp")
            nc.tensor.transpose(out=ohTp[:], in_=onehot[:], identity=ident_r[:])
            ohT = sb.tile([P, P], F32R, tag="ohT")
            nc.scalar.copy(ohT[:], ohTp[:])
            xrg = psx.tile([P, HC3], F32, space="PSUM", tag="xrg")
            nc.tensor.matmul(xrg[:, :w], lhsT=ident_r[:], rhs=xl[:],
                             start=True, stop=False)
            nc.tensor.matmul(xrg[:, :w], lhsT=ohT[:],
                             rhs=xr_slot[:, :w], start=False, stop=True)
            g = sb.tile([P, w], F32, tag=f"g{w}")
            nc.scalar.activation(g[:], xrg[:, :w], AF.Prelu, alpha=0.2)
            ch = w // H
            alpha4 = sb.tile([P, H], F32, tag="alpha4")
            junk = sb.tile([P, w // H], F32, tag="junk")
            for h in range(H):
                sl = slice(h * ch, (h + 1) * ch)
                nc.vector.scalar_tensor_tensor(
                    out=junk[:, :ch], in0=g[:, sl], scalar=1.0,
                    in1=att_t[:, sl], op0=OP.mult, op1=OP.mult,
                    accum_out=alpha4[:, h:h + 1])
            msgs = sb.tile([P, w + H], F32R, tag=f"msgs{w}")
            nc.scalar.activation(msgs[:, w:w + H], alpha4[:], AF.Exp)
            # msgs[:, :w] = xl * ea (per-head broadcast) in one 3D TT
            nc.vector.tensor_tensor(
                out=_ap3(msgs[:], H, ch, ch, 1),
                in0=_ap3(xl[:].bitcast(F32), H, ch, ch, 1),
                in1=_ap3(msgs[:, w:w + H], H, ch, 1, 0),
                op=OP.mult)
            if w == HC1:
                nc.tensor.matmul(psum_m[:], lhsT=onehot[:],
                                 rhs=msgs[:, :w + H], start=first,
                                 stop=last)
            else:
                nc.tensor.matmul(psum_m[:], lhsT=onehot[:],
                                 rhs=msgs[:, :w], start=first, stop=last)
                nc.tensor.matmul(psum_d[:], lhsT=onehot[:],
                                 rhs=msgs[:, w:w + H], start=first,
                                 stop=last)

        def norm_heads(psum_ap, den_ap, w, tag):
            """y[:, h*ch:(h+1)*ch] = psum_h / (den_h + 1e-16)."""
            ch = w // H
            den = sb.tile([P, H], F32, tag="den")
            nc.vector.tensor_scalar(out=den[:], in0=den_ap, scalar1=1e-16,
                                    scalar2=None, op0=OP.add)
            rden = sb.tile([P, H], F32, tag="rden")
            nc.vector.reciprocal(rden[:], den[:])
            y = ev.tile([P, w], F32, tag=tag)
            nc.vector.tensor_tensor(
                out=_ap3(y[:], H, ch, ch, 1),
                in0=_ap3(psum_ap, H, ch, ch, 1),
                in1=_ap3(rden[:], H, ch, 1, 0),
                op=OP.mult)
            return y, rden

        # ---- phase B: sg edges -> x1_gene -> xl3 rows, AG2
        ci = 0
        for slot in range(G_PER_CORE):
            xr_slot0 = sb.tile([P, HC1], F32, tag="xrslot1a")
            nc.sync.dma_start(xr_slot0[:], xr1_sg[slot * P:(slot + 1) * P, :])
            xr_slot = sb.tile([P, HC1], F32R, tag="xrslot1")
            nc.scalar.copy(xr_slot[:], xr_slot0[:])
            pm = pse.tile([P, HC1 + H], F32, space="PSUM", tag="pm")
            for k in range(int(g_sched[slot])):
                edge_chunk(ci, sg_srcr_t, sg_dstl_t, tbl_s,
                           xr_slot, att1_sg_t, pm, None,
                           k == 0, k == int(g_sched[slot]) - 1, HC1)
                ci += 1
            y, _ = norm_heads(pm[:], pm[:, HC1:HC1 + H], HC1, "y1g")
            y2 = ev.tile([P, HC1], F32, tag="y2g")
            nc.vector.tensor_tensor(out=y2[:], in0=y[:], in1=bias1_sg_t[:],
                                    op=OP.add)
            x1 = ev.tile([P, HC1], F32, tag="x1g")
            _elu(nc, ev, x1[:], y2[:], HC1)
            xT = _transpose2(nc, sb, psp, ident, x1[:], HC1 // P)
            dense_out(xT, Wl3_t, HC3, bl3_t,
                      dst_ap=agin_3[slot * P:(slot + 1) * P, :], rdt=F32R)
        nc.gpsimd.collective_compute("AllGather", OP.bypass, replica_groups=RG,
                                     ins=[agin_3[:]], outs=[tbl_3[:]])

        # ---- phase C: gs edges -> x1_sample -> xr3/sl3 rows
        ci = 0
        for slot in range(S_PER_CORE):
            xr_slot0 = sb.tile([P, HC1], F32, tag="xrslot1a")
            nc.sync.dma_start(xr_slot0[:], xr1_gs[slot * P:(slot + 1) * P, :])
            xr_slot = sb.tile([P, HC1], F32R, tag="xrslot1")
            nc.scalar.copy(xr_slot[:], xr_slot0[:])
            pm = pse.tile([P, HC1 + H], F32, space="PSUM", tag="pm")
            for k in range(int(s_sched[slot])):
                edge_chunk(ci, gs_srcr_t, gs_dstl_t, tbl_g,
                           xr_slot, att1_gs_t, pm, None,
                           k == 0, k == int(s_sched[slot]) - 1, HC1)
                ci += 1
            y, _ = norm_heads(pm[:], pm[:, HC1:HC1 + H], HC1, "y1s")
            y2 = ev.tile([P, HC1], F32, tag="y2s")
            nc.vector.tensor_tensor(out=y2[:], in0=y[:], in1=bias1_gs_t[:],
                                    op=OP.add)
            y3 = ev.tile([P, HC1], F32, tag="y3s")
            sl1_ap = bass.AP(sl1_sb.tensor,
                             sl1_sb[:, slot * C1:(slot + 1) * C1].offset,
                             [[sl1_sb[:].ap[0][0], P], [0, H], [1, C1]])
            y2v = bass.AP(y2.tensor, y2[:].offset,
                          [[y2[:].ap[0][0], P], [C1, H], [1, C1]])
            y3v = bass.AP(y3.tensor, y3[:].offset,
                          [[y3[:].ap[0][0], P], [C1, H], [1, C1]])
            nc.vector.tensor_tensor(out=y3v, in0=y2v, in1=sl1_ap, op=OP.add)
            x1 = ev.tile([P, HC1], F32, tag="x1s")
            _elu(nc, ev, x1[:], y3[:], HC1)
            xT = _transpose2(nc, sb, psp, ident, x1[:], HC1 // P)
            dense_out(xT, Wr3_t, HC3, br3_t,
                      dst_ap=xr3[slot * P:(slot + 1) * P, :])
            dense_out(xT, sl3_W_t, C3, sl3_b_t,
                      sbuf_dst=sl3_sb[:, slot * C3:(slot + 1) * C3])

        # ---- phase D: gs edges layer 3 -> output
        ci = 0
        for slot in range(S_PER_CORE):
            xr_slot0 = sb.tile([P, HC3], F32, tag="xrslot3a")
            nc.sync.dma_start(xr_slot0[:], xr3[slot * P:(slot + 1) * P, :])
            xr_slot = sb.tile([P, HC3], F32R, tag="xrslot3")
            nc.scalar.copy(xr_slot[:], xr_slot0[:])
            pm = pse.tile([P, HC3], F32, space="PSUM", tag="pm")
            pd = ps1.tile([P, H], F32, space="PSUM", tag="aux")
            for k in range(int(s_sched[slot])):
                edge_chunk(ci, gs_srcr3_t, gs_dstl_t, tbl_3,
                           xr_slot, att3_t, pm, pd,
                           k == 0, k == int(s_sched[slot]) - 1, HC3)
                ci += 1
            # mean over heads of psum_h / den_h == sum_h psum_h * (0.25/den_h)
            den4 = sb.tile([P, H], F32, tag="den")
            nc.vector.tensor_scalar(out=den4[:], in0=pd[:], scalar1=4.0,
                                    scalar2=4e-16, op0=OP.mult, op1=OP.add)
            rden = sb.tile([P, H], F32, tag="rden")
            nc.vector.reciprocal(rden[:], den4[:])
            base = ev.tile([P, C3], F32, tag="based")
            nc.vector.tensor_tensor(out=base[:],
                                    in0=sl3_sb[:, slot * C3:(slot + 1) * C3],
                                    in1=bias3_t[:], op=OP.add)
            accs = [base]
            for h in range(H):
                a = ev.tile([P, C3], F32, tag=f"acc{h}")
                nc.vector.scalar_tensor_tensor(
                    out=a[:], in0=pm[:, h * C3:(h + 1) * C3],
                    scalar=rden[:, h:h + 1], in1=accs[-1][:],
                    op0=OP.mult, op1=OP.add)
                accs.append(a)
            o = ev.tile([P, C3], F32, tag="outt")
            _elu(nc, ev, o[:], accs[-1][:], C3)
            nc.sync.dma_start(out_own[slot * P:(slot + 1) * P, :], o[:])

    nc.compile()
    return nc


# ------------------------------------------------------------------ driver

_CACHE = {}


def kernel(**inputs):
    plan, in_maps = _plan(inputs)
    key = (tuple(plan["g_sched"]), tuple(plan["s_sched"]))
    if key not in _CACHE:
        _CACHE[key] = _build(plan["g_sched"], plan["s_sched"])
    nc = _CACHE[key]
    r = run_bass_kernel_spmd(nc, in_maps, core_ids=list(range(NCORES)))
    out = np.zeros((NS, C3), np.float32)
    s_assign = plan["s_assign"]
    for c in range(NCORES):
        oc = r.results[c]["out_own"]
        for slot in range(S_PER_CORE):
            t = s_assign[slot, c]
            out[t * P:(t + 1) * P] = oc[slot * P:(slot + 1) * P]
    return out



# revision 16
# speedup vs baseline: 1.0176x; 1.0176x over previous
"""BiRGAT (bipartite 2-layer GATv2) Trainium2 kernel, 8-core SPMD.

Strategy: destination-tile sharding (as before), with a bf16 batched edge
pipeline. Gene dsts padded to 160 tiles of 128 (20/core), sample dsts 32
tiles (4/core), dealt to cores by sorted chunk count. Source-side node
tables are bf16 in DRAM, AllGathered between phases.

Edge phase per slot, in groups of <=8 128-edge chunks:
- ONE batched indirect gather of source rows [128, G, w] (amortizes the
  ~1us SWDGE descriptor-gen cost on the Pool engine).
- one-hot per chunk via a single TensorScalarPtr (bf16, 4x DVE mode).
- PE transposes -> batched PSUM->SBUF copy gives ohT (bf16).
- z = ident@xl + ohT@xr on PE (PSUM fp32), Prelu on Act (2-chunk batch).
- alpha: batched (g * att_b) TT (bf16 2x), log2 fold adds 64->8, one
  tensor_reduce to [128, G*H] fp32, batched Exp.
- messages: layer1 = xl * ea broadcast (one TT per group) + scatter
  matmul [w+H] (ea cols give the denominator); layer3 = per-head
  ea-scaled one-hots (TSPtr is_equal*mult) + per-head scatter matmuls.
Tile evacuation does softmax normalization (no segment_max; alpha std
~0.3 so den+1e-16 makes max-subtraction irrelevant), bias, ELU, residuals.
"""
import sys

sys.path.insert(0, "/opt/trn_rl_repo")

import numpy as np
import ml_dtypes
from contextlib import ExitStack

import concourse.bass as bass
import concourse.tile as tile
from concourse import bacc, mybir
from concourse.bass_utils import run_bass_kernel_spmd
from concourse.masks import make_identity

P = 128
NCORES = 8
NS, NG, E = 4096, 20000, 131072
DIN, H, C1, C3 = 256, 4, 64, 128
HC1, HC3 = H * C1, H * C3          # 256, 512
NGP = 20480                        # genes padded to 160 tiles
NST, NGT = NS // P, NGP // P       # 32, 160
S_PER_CORE, G_PER_CORE = NST // NCORES, NGT // NCORES   # 4, 20
SROWS, GROWS = S_PER_CORE * P, G_PER_CORE * P           # 512, 2560

F32 = mybir.dt.float32
BF16 = mybir.dt.bfloat16
I32 = mybir.dt.int32
I16 = mybir.dt.int16
AF = mybir.ActivationFunctionType
OP = mybir.AluOpType
AX = mybir.AxisListType
BF = ml_dtypes.bfloat16

PAD_LOC = 200.0   # dst-local sentinel for padded edges (never equals 0..127)
GMAX = 8          # chunks per gather/DVE batch group
import os
SIM_SAFE = os.environ.get("BIRGAT_SIM_SAFE", "0") == "1"  # CoreSim lacks Prelu


# ---------------------------------------------------------------- host plan

def _deal_tiles(dst, n_tiles, per_core):
    tcnt = np.bincount(dst // P, minlength=n_tiles)
    chunks = np.maximum((tcnt + P - 1) // P, 1)
    order = np.argsort(-chunks, kind="stable")
    assign = order.reshape(per_core, NCORES)
    sched = chunks[assign].max(axis=1)
    return assign, sched.astype(int), tcnt


def _edge_arrays(src, dst, assign, sched, src_row_map, core):
    """Per-core edge chunk arrays for one relation.
    Returns src_rows [P, NCH] i32, dstloc [P, NCH] bf16."""
    nch = int(sched.sum())
    src_rows = np.zeros((nch, P), np.int32)
    dstloc = np.full((nch, P), PAD_LOC, np.float32)
    tile_of = dst // P
    ci = 0
    for slot in range(len(sched)):
        t = assign[slot, core]
        e = np.nonzero(tile_of == t)[0]
        n = len(e)
        want = sched[slot] * P
        s = np.zeros(want, np.int32)
        dl = np.full(want, PAD_LOC, np.float32)
        s[:n] = src_row_map[src[e]]
        dl[:n] = (dst[e] % P).astype(np.float32)
        src_rows[ci:ci + sched[slot]] = s.reshape(-1, P)
        dstloc[ci:ci + sched[slot]] = dl.reshape(-1, P)
        ci += sched[slot]
    idx16 = np.tile(src_rows.reshape(-1, 16).T.astype(np.int16), (8, 1))
    return idx16.copy(), dstloc.T.copy()


def _bcast(v, dt=np.float32, p=P):
    return np.broadcast_to(np.asarray(v, np.float32).reshape(1, -1),
                           (p, len(np.asarray(v).reshape(-1)))).astype(dt)


def _wtile(w, kdim, n):
    """Host-side [128, kdim//128, n] bf16 rhs layout for matmul weights."""
    a = np.asarray(w, np.float32).reshape(kdim // P, P, n)
    return a.transpose(1, 0, 2).astype(BF).copy()


def _xT(x, rows):
    """Host-side [128, DIN//128, rows] bf16 transposed-feature staging."""
    a = np.zeros((DIN, rows), np.float32)
    a[:, :x.shape[0]] = np.asarray(x, np.float32).T
    return a.reshape(DIN // P, P, rows).transpose(1, 0, 2).astype(BF).copy()


def _plan(inputs):
    sg_src = np.asarray(inputs["sg_src"]); sg_dst = np.asarray(inputs["sg_dst"])
    gs_src = np.asarray(inputs["gs_src"]); gs_dst = np.asarray(inputs["gs_dst"])

    g_assign, g_sched, _ = _deal_tiles(sg_dst, NGT, G_PER_CORE)
    s_assign, s_sched, _ = _deal_tiles(gs_dst, NST, S_PER_CORE)

    g_owner = np.zeros(NGT, np.int32); g_slot = np.zeros(NGT, np.int32)
    for slot in range(G_PER_CORE):
        for c in range(NCORES):
            g_owner[g_assign[slot, c]] = c
            g_slot[g_assign[slot, c]] = slot
    s_owner = np.zeros(NST, np.int32); s_slot = np.zeros(NST, np.int32)
    for slot in range(S_PER_CORE):
        for c in range(NCORES):
            s_owner[s_assign[slot, c]] = c
            s_slot[s_assign[slot, c]] = slot

    sid = np.arange(NS)
    srow_tbl = s_owner[sid // P] * SROWS + s_slot[sid // P] * P + sid % P
    gid = np.arange(NG)
    grow_tbl = g_owner[gid // P] * GROWS + g_slot[gid // P] * P + gid % P

    plan = {
        "g_assign": g_assign, "g_sched": g_sched,
        "s_assign": s_assign, "s_sched": s_sched,
    }

    x_sample = np.asarray(inputs["x_sample"], np.float32)
    x_gene = np.asarray(inputs["x_gene"], np.float32)

    in_maps = []
    for c in range(NCORES):
        s_tiles = s_assign[:, c]
        xs_own = x_sample.reshape(NST, P, DIN)[s_tiles].reshape(SROWS, DIN)
        g_tiles = g_assign[:, c]
        xg_own = np.zeros((GROWS, DIN), np.float32)
        for i, t in enumerate(g_tiles):
            lo = t * P
            if lo < NG:
                n = min(P, NG - lo)
                xg_own[i * P:i * P + n] = x_gene[lo:lo + n]

        sgS, sgL = _edge_arrays(sg_src, sg_dst, g_assign, g_sched, srow_tbl, c)
        gsS, gsL = _edge_arrays(gs_src, gs_dst, s_assign, s_sched, grow_tbl, c)

        m = {
            "xsT_own": _xT(xs_own, SROWS),
            "xgT_own": _xT(xg_own, GROWS),
            "Wl1_sg": _wtile(inputs["Wl1_sg"], DIN, HC1),
            "Wr1_sg": _wtile(inputs["Wr1_sg"], DIN, HC1),
            "Wl1_gs": _wtile(inputs["Wl1_gs"], DIN, HC1),
            "Wr1_gs": _wtile(inputs["Wr1_gs"], DIN, HC1),
            "Wl3": _wtile(inputs["Wl3_gs"], HC1, HC3),
            "Wr3": _wtile(inputs["Wr3_gs"], HC1, HC3),
            "sl1_W": _wtile(inputs["sl1_W"], DIN, C1),
            "sl3_W": _wtile(inputs["sl3_W"], HC1, C3),
            "att1_sg_b": _bcast(np.asarray(inputs["att1_sg"]).reshape(-1), BF),
            "att1_gs_b": _bcast(np.asarray(inputs["att1_gs"]).reshape(-1), BF),
            "att3_b": _bcast(np.asarray(inputs["att3_gs"]).reshape(-1), BF),
            "bl1_sg_b": _bcast(inputs["bl1_sg"]),
            "br1_sg_b": _bcast(inputs["br1_sg"]),
            "bl1_gs_b": _bcast(inputs["bl1_gs"]),
            "br1_gs_b": _bcast(inputs["br1_gs"]),
            "bias1_sg_b": _bcast(inputs["bias1_sg"]),
            "bias1_gs_b": _bcast(inputs["bias1_gs"]),
            "bl3_b": _bcast(inputs["bl3_gs"]),
            "br3_b": _bcast(inputs["br3_gs"]),
            "bias3_b": _bcast(inputs["bias3_gs"]),
            "sl1_b_b": _bcast(inputs["sl1_b"]),
            "sl3_b_b": _bcast(inputs["sl3_b"]),
            "sg_idx": sgS, "sg_dstl": sgL,
            "gs_idx": gsS, "gs_dstl": gsL,
        }
        in_maps.append(m)
    return plan, in_maps


# ------------------------------------------------------------- device build

def _groups(nch):
    out = []
    ci = 0
    while ci < nch:
        g = min(GMAX, nch - ci)
        out.append((ci, g))
        ci += g
    return out


def _ap(base, offset_cols, shape_strides):
    """AP over base tile's tensor: shape_strides = [[stride, n], ...] free."""
    a = base[:]
    return bass.AP(a.tensor, a.offset + offset_cols,
                   [[a.ap[0][0], P]] + shape_strides)


def _build(g_sched, s_sched):
    nsg = int(g_sched.sum())
    ngs = int(s_sched.sum())
    nc = bacc.Bacc("TRN2", target_bir_lowering=False, debug=False,
                   num_devices=NCORES)

    ei = lambda name, shape, dt=F32: nc.dram_tensor(name, shape, dt,
                                                    kind="ExternalInput")
    xsT_own = ei("xsT_own", [P, DIN // P, SROWS], BF16)
    xgT_own = ei("xgT_own", [P, DIN // P, GROWS], BF16)
    Wl1_sg = ei("Wl1_sg", [P, 2, HC1], BF16); Wr1_sg = ei("Wr1_sg", [P, 2, HC1], BF16)
    Wl1_gs = ei("Wl1_gs", [P, 2, HC1], BF16); Wr1_gs = ei("Wr1_gs", [P, 2, HC1], BF16)
    Wl3 = ei("Wl3", [P, 2, HC3], BF16); Wr3 = ei("Wr3", [P, 2, HC3], BF16)
    sl1_W = ei("sl1_W", [P, 2, C1], BF16); sl3_W = ei("sl3_W", [P, 2, C3], BF16)
    att1_sg_b = ei("att1_sg_b", [P, HC1], BF16)
    att1_gs_b = ei("att1_gs_b", [P, HC1], BF16)
    att3_b = ei("att3_b", [P, HC3], BF16)
    bl1_sg_b = ei("bl1_sg_b", [P, HC1]); br1_sg_b = ei("br1_sg_b", [P, HC1])
    bl1_gs_b = ei("bl1_gs_b", [P, HC1]); br1_gs_b = ei("br1_gs_b", [P, HC1])
    bias1_sg_b = ei("bias1_sg_b", [P, HC1]); bias1_gs_b = ei("bias1_gs_b", [P, HC1])
    bl3_b = ei("bl3_b", [P, HC3]); br3_b = ei("br3_b", [P, HC3])
    bias3_b = ei("bias3_b", [P, C3])
    sl1_b_b = ei("sl1_b_b", [P, C1]); sl3_b_b = ei("sl3_b_b", [P, C3])
    sg_idx = ei("sg_idx", [P, nsg * 8], I16)
    sg_dstl = ei("sg_dstl", [P, nsg])
    gs_idx = ei("gs_idx", [P, ngs * 8], I16)
    gs_dstl = ei("gs_dstl", [P, ngs])

    out_own = nc.dram_tensor("out_own", [SROWS, C3], F32, kind="ExternalOutput")

    # DRAM scratch (tables bf16)
    agin_s = nc.dram_tensor("agin_s", [SROWS, HC1], BF16)
    agin_g = nc.dram_tensor("agin_g", [GROWS, HC1], BF16)
    agin_3 = nc.dram_tensor("agin_3", [GROWS, HC3], BF16)
    tbl_s = nc.dram_tensor("tbl_s", [NS, HC1], BF16, addr_space="Shared")
    tbl_g = nc.dram_tensor("tbl_g", [NGP, HC1], BF16, addr_space="Shared")
    tbl_3 = nc.dram_tensor("tbl_3", [NGP, HC3], BF16, addr_space="Shared")
    xr1_sg = nc.dram_tensor("xr1_sg", [GROWS, HC1], BF16)
    xr1_gs = nc.dram_tensor("xr1_gs", [SROWS, HC1], BF16)
    xr3 = nc.dram_tensor("xr3", [SROWS, HC3], BF16)

    RG = [list(range(NCORES))]

    with tile.TileContext(nc) as tc, ExitStack() as ctx:
        res = ctx.enter_context(tc.tile_pool(name="res", bufs=1))
        sb = ctx.enter_context(tc.tile_pool(name="sb", bufs=2))
        ev = ctx.enter_context(tc.tile_pool(name="ev", bufs=2))
        psz = ctx.enter_context(tc.tile_pool(name="psz", bufs=2, space="PSUM"))
        psoh = ctx.enter_context(tc.tile_pool(name="psoh", bufs=2, space="PSUM"))
        pse = ctx.enter_context(tc.tile_pool(name="pse", bufs=2, space="PSUM"))
        ps1 = ctx.enter_context(tc.tile_pool(name="ps1", bufs=1, space="PSUM"))

        ident = res.tile([P, P], F32)
        make_identity(nc, ident[:])
        ident_bf = res.tile([P, P], BF16)
        nc.scalar.copy(ident_bf[:], ident[:])
        iota_bf = res.tile([P, P], BF16)
        nc.gpsimd.iota(iota_bf[:], pattern=[[1, P]], base=0,
                       channel_multiplier=0,
                       allow_small_or_imprecise_dtypes=True)
        ones_bf = res.tile([P, 1], BF16)
        nc.vector.memset(ones_bf[:], 1.0)

        def rload(name, dram, shape, dt=F32):
            t = res.tile(shape, dt, tag=name)
            nc.sync.dma_start(t[:], dram[:])
            return t

        att1_sg_t = rload("a1s", att1_sg_b, [P, HC1], BF16)
        att1_gs_t = rload("a1g", att1_gs_b, [P, HC1], BF16)
        att3_t = rload("a3", att3_b, [P, HC3], BF16)
        bias1_sg_t = rload("b1s", bias1_sg_b, [P, HC1])
        bias1_gs_t = rload("b1g", bias1_gs_b, [P, HC1])
        bias3_t = rload("b3", bias3_b, [P, C3])
        sg_idx_t = rload("sgs", sg_idx, [P, nsg * 8], I16)
        sg_dstl_t = rload("sgl", sg_dstl, [P, nsg])
        gs_idx_t = rload("gss", gs_idx, [P, ngs * 8], I16)
        gs_dstl_t = rload("gsl", gs_dstl, [P, ngs])
        xsT = rload("xsT", xsT_own, [P, DIN // P, SROWS], BF16)
        xgT = rload("xgT", xgT_own, [P, DIN // P, GROWS], BF16)

        Wl1_sg_t = rload("Wl1_sg", Wl1_sg, [P, 2, HC1], BF16)
        Wr1_sg_t = rload("Wr1_sg", Wr1_sg, [P, 2, HC1], BF16)
        Wl1_gs_t = rload("Wl1_gs", Wl1_gs, [P, 2, HC1], BF16)
        Wr1_gs_t = rload("Wr1_gs", Wr1_gs, [P, 2, HC1], BF16)
        Wl3_t = rload("Wl3", Wl3, [P, 2, HC3], BF16)
        Wr3_t = rload("Wr3", Wr3, [P, 2, HC3], BF16)
        sl1_W_t = rload("sl1_W", sl1_W, [P, 2, C1], BF16)
        sl3_W_t = rload("sl3_W", sl3_W, [P, 2, C3], BF16)
        bl1_sg_t = rload("bl1s", bl1_sg_b, [P, HC1])
        br1_sg_t = rload("br1s", br1_sg_b, [P, HC1])
        bl1_gs_t = rload("bl1g", bl1_gs_b, [P, HC1])
        br1_gs_t = rload("br1g", br1_gs_b, [P, HC1])
        bl3_t = rload("bl3", bl3_b, [P, HC3])
        br3_t = rload("br3", br3_b, [P, HC3])
        sl1_b_t = rload("sl1b", sl1_b_b, [P, C1])
        sl3_b_t = rload("sl3b", sl3_b_b, [P, C3])

        sl1_sb = res.tile([P, S_PER_CORE * C1], F32)
        sl3_sb = res.tile([P, S_PER_CORE * C3], F32)

        def dense_T(lhsT, w_sb, n, bias_t, dst_ap=None, sbuf_dst=None,
                    rdt=BF16, tag="dout"):
            """out = lhsT.T @ W + bias; lhsT = list of [128,128] bf16 APs."""
            pt = ps1.tile([P, HC3], F32, space="PSUM", tag="aux")
            for k in range(len(lhsT)):
                nc.tensor.matmul(pt[:, :n], lhsT=lhsT[k], rhs=w_sb[:, k, :n],
                                 start=(k == 0), stop=(k == len(lhsT) - 1))
            o = sbuf_dst
            if o is None:
                ot = sb.tile([P, n], rdt, tag=tag)
                o = ot[:]
            nc.vector.tensor_tensor(out=o, in0=pt[:, :n], in1=bias_t[:, :n],
                                    op=OP.add)
            if dst_ap is not None:
                nc.sync.dma_start(dst_ap, ot[:])
            return o

        # ---- phase A: node tables (no on-chip transposes; xT staged)
        for i in range(S_PER_CORE):
            lhsT = [xsT[:, k, i * P:(i + 1) * P] for k in range(DIN // P)]
            dense_T(lhsT, Wl1_sg_t, HC1, bl1_sg_t,
                    dst_ap=agin_s[i * P:(i + 1) * P, :])
            dense_T(lhsT, Wr1_gs_t, HC1, br1_gs_t,
                    dst_ap=xr1_gs[i * P:(i + 1) * P, :])
            dense_T(lhsT, sl1_W_t, C1, sl1_b_t,
                    sbuf_dst=sl1_sb[:, i * C1:(i + 1) * C1])
        nc.gpsimd.collective_compute("AllGather", OP.bypass, replica_groups=RG,
                                     ins=[agin_s[:]], outs=[tbl_s[:]])

        for j in range(G_PER_CORE):
            lhsT = [xgT[:, k, j * P:(j + 1) * P] for k in range(DIN // P)]
            dense_T(lhsT, Wl1_gs_t, HC1, bl1_gs_t,
                    dst_ap=agin_g[j * P:(j + 1) * P, :])
            dense_T(lhsT, Wr1_sg_t, HC1, br1_sg_t,
                    dst_ap=xr1_sg[j * P:(j + 1) * P, :])
        nc.gpsimd.collective_compute("AllGather", OP.bypass, replica_groups=RG,
                                     ins=[agin_g[:]], outs=[tbl_g[:]])

        # ---- edge slot processing
        def edge_slot(ci0, nch, idx_t, dstl_t, tbl, xr_sb, att_t, w, pm, pden):
            """Process one dst slot's nch chunks; accumulate into pm (+pden
            for layer3 mode, which is signalled by pden is not None)."""
            l3 = pden is not None
            nfold = 4 if w == HC3 else 3   # per-head 128->8 or 64->8
            ch = w // H
            for gi, (goff, G) in enumerate(_groups(nch)):
                ci = ci0 + goff
                first0 = goff == 0
                # batched gather [128, G, w]
                xl = sb.tile([P, GMAX, w], BF16, tag=f"xl{w}")
                nc.gpsimd.dma_gather(
                    out_ap=_ap(xl, 0, [[w, G], [1, w]]), in_ap=tbl[:],
                    idxs_ap=idx_t[:, ci * 8:(ci + G) * 8],
                    num_idxs=G * P, num_idxs_reg=G * P, elem_size=w)
                # one-hots (bf16, 4x TSPtr) + PE transposes -> ohT bf16
                onehot = sb.tile([P, GMAX, P], BF16, tag="onehot")
                ohT = sb.tile([P, GMAX, P], BF16, tag="ohT")
                for g in range(G):
                    nc.vector.tensor_scalar(
                        out=onehot[:, g, :], in0=iota_bf[:],
                        scalar1=dstl_t[:, ci + g:ci + g + 1], scalar2=None,
                        op0=OP.is_equal)
                for half in range(0, G, 4):
                    hn = min(4, G - half)
                    pt = psoh.tile([P, 4, P], BF16, space="PSUM", tag="ohTp")
                    for g in range(hn):
                        nc.tensor.transpose(out=pt[:, g, :],
                                            in_=onehot[:, half + g, :],
                                            identity=ident_bf[:])
                    nc.scalar.activation(
                        _ap(ohT, half * P, [[P, hn], [1, P]]),
                        _ap(pt, 0, [[P, hn], [1, P]]), AF.Copy)
                # z = ident@xl + ohT@xr (PSUM fp32), Prelu -> g bf16
                gt = sb.tile([P, GMAX, w], BF16, tag=f"g{w}")
                zb = 1 if l3 else 2   # chunks per PSUM z tile (2KB bank)
                for z0 in range(0, G, zb):
                    zn = min(zb, G - z0)
                    zp = psz.tile([P, HC3], F32, space="PSUM", tag="z")
                    for g in range(z0, z0 + zn):
                        zv = _ap(zp, (g - z0) * w, [[1, w]])
                        nc.tensor.matmul(zv, lhsT=ident_bf[:],
                                         rhs=xl[:, g, :], start=True,
                                         stop=False)
                        nc.tensor.matmul(zv, lhsT=ohT[:, g, :],
                                         rhs=xr_sb[:, :w], start=False,
                                         stop=True)
                    if SIM_SAFE:
                        nc.vector.scalar_tensor_tensor(
                            out=_ap(gt, z0 * w, [[w, zn], [1, w]]),
                            in0=_ap(zp, 0, [[w, zn], [1, w]]), scalar=0.2,
                            in1=_ap(zp, 0, [[w, zn], [1, w]]),
                            op0=OP.mult, op1=OP.max)
                    else:
                        nc.scalar.activation(
                            _ap(gt, z0 * w, [[w, zn], [1, w]]),
                            _ap(zp, 0, [[w, zn], [1, w]]), AF.Prelu, alpha=0.2)
                # alpha: gm = g*att (bf16 2x), fold 64/128 -> 8, reduce, exp
                gm = sb.tile([P, GMAX, w], BF16, tag=f"gm{w}")
                nc.vector.tensor_tensor(
                    out=_ap(gm, 0, [[w, G], [1, w]]),
                    in0=_ap(gt, 0, [[w, G], [1, w]]),
                    in1=_ap(att_t, 0, [[0, G], [1, w]]), op=OP.mult)
                src_t, src_w, src_off = gm, w, 0
                for f in range(nfold):
                    hw_ = src_w // H // 2       # half of per-head width
                    ft = sb.tile([P, GMAX * H * hw_], BF16, tag=f"f{w}_{f}")
                    nc.vector.tensor_tensor(
                        out=_ap(ft, 0, [[hw_, G * H], [1, hw_]]),
                        in0=_ap(src_t, src_off, [[2 * hw_, G * H], [1, hw_]]),
                        in1=_ap(src_t, src_off + hw_,
                                [[2 * hw_, G * H], [1, hw_]]),
                        op=OP.add)
                    src_t, src_w, src_off = ft, src_w // 2, 0
                alpha = sb.tile([P, GMAX * H], F32, tag="alpha")
                nc.vector.tensor_reduce(
                    out=_ap(alpha, 0, [[1, G * H]]),
                    in_=_ap(src_t, 0, [[8, G * H], [1, 8]]),
                    axis=AX.X, op=OP.add)
                msgs = sb.tile([P, GMAX, w + H], BF16, tag=f"msgs{w}")
                nc.scalar.activation(
                    _ap(msgs, w, [[w + H, G], [1, H]]),
                    _ap(alpha, 0, [[H, G], [1, H]]), AF.Exp)
                nc.vector.tensor_tensor(
                    out=_ap(msgs, 0, [[w + H, G], [ch, H], [1, ch]]),
                    in0=_ap(xl, 0, [[w, G], [ch, H], [1, ch]]),
                    in1=_ap(msgs, w, [[w + H, G], [1, H], [0, ch]]),
                    op=OP.mult)
                for g in range(G):
                    st = first0 and g == 0
                    sp = goff + g == nch - 1
                    if l3:
                        nc.tensor.matmul(pm[:, :w], lhsT=onehot[:, g, :],
                                         rhs=msgs[:, g, :w], start=st, stop=sp)
                        nc.tensor.matmul(pden[:, :H], lhsT=onehot[:, g, :],
                                         rhs=msgs[:, g, w:w + H],
                                         start=st, stop=sp)
                    else:
                        nc.tensor.matmul(pm[:, :w + H], lhsT=onehot[:, g, :],
                                         rhs=msgs[:, g, :], start=st, stop=sp)

        def norm_heads(psum_ap, den_ap, w, tag):
            ch = w // H
            den = ev.tile([P, H], F32, tag="den")
            nc.vector.tensor_scalar(out=den[:], in0=den_ap, scalar1=1e-16,
                                    scalar2=None, op0=OP.add)
            rden = ev.tile([P, H], F32, tag="rden")
            nc.vector.reciprocal(rden[:], den[:])
            y = ev.tile([P, w], F32, tag=tag)
            nc.vector.tensor_tensor(
                out=_ap(y, 0, [[ch, H], [1, ch]]),
                in0=psum_ap,
                in1=_ap(rden, 0, [[1, H], [0, ch]]),
                op=OP.mult)
            return y

        def _elu(out_ap, y_ap, w):
            m = ev.tile([P, w], F32, tag="elu_m")
            nc.vector.tensor_scalar(out=m[:], in0=y_ap, scalar1=0.0,
                                    scalar2=None, op0=OP.min)
            e = ev.tile([P, w], F32, tag="elu_e")
            nc.scalar.activation(e[:], m[:], AF.Exp)
            r = ev.tile([P, w], F32, tag="elu_r")
            nc.scalar.activation(r[:], y_ap, AF.Relu)
            nc.vector.scalar_tensor_tensor(out=out_ap, in0=r[:], scalar=-1.0,
                                           in1=e[:], op0=OP.add, op1=OP.add)

        def x1_transposed(x1_ap, tag):
            """fp32 x1 [128, HC1] -> bf16, PE-transpose -> 2 bf16 lhsT."""
            xb = ev.tile([P, HC1], BF16, tag=tag + "b")
            nc.scalar.activation(xb[:], x1_ap, AF.Copy)
            pt = psoh.tile([P, 4, P], BF16, space="PSUM", tag="ohTp")
            for k in range(2):
                nc.tensor.transpose(out=pt[:, k, :], in_=xb[:, k * P:(k + 1) * P],
                                    identity=ident_bf[:])
            xT2 = ev.tile([P, 2, P], BF16, tag=tag + "T")
            nc.scalar.activation(_ap(xT2, 0, [[P, 2], [1, P]]),
                                 _ap(pt, 0, [[P, 2], [1, P]]), AF.Copy)
            return [xT2[:, 0, :], xT2[:, 1, :]]

        # ---- phase B: sg edges -> x1_gene -> xl3 rows, AG2
        ci = 0
        for slot in range(G_PER_CORE):
            xr_sb = sb.tile([P, HC1], BF16, tag="xr1")
            nc.sync.dma_start(xr_sb[:], xr1_sg[slot * P:(slot + 1) * P, :])
            pm = pse.tile([P, HC3], F32, space="PSUM", tag="pm")
            edge_slot(ci, int(g_sched[slot]), sg_idx_t, sg_dstl_t, tbl_s,
                      xr_sb, att1_sg_t, HC1, pm, None)
            ci += int(g_sched[slot])
            y = norm_heads(pm[:, :HC1], pm[:, HC1:HC1 + H], HC1, "y1g")
            y2 = ev.tile([P, HC1], F32, tag="y2g")
            nc.vector.tensor_tensor(out=y2[:], in0=y[:], in1=bias1_sg_t[:],
                                    op=OP.add)
            x1 = ev.tile([P, HC1], F32, tag="x1g")
            _elu(x1[:], y2[:], HC1)
            lhsT = x1_transposed(x1[:], "x1gT")
            dense_T(lhsT, Wl3_t, HC3, bl3_t,
                    dst_ap=agin_3[slot * P:(slot + 1) * P, :])
        nc.gpsimd.collective_compute("AllGather", OP.bypass, replica_groups=RG,
                                     ins=[agin_3[:]], outs=[tbl_3[:]])

        # ---- phase C: gs edges -> x1_sample -> xr3/sl3 rows
        ci = 0
        for slot in range(S_PER_CORE):
            xr_sb = sb.tile([P, HC1], BF16, tag="xr1")
            nc.sync.dma_start(xr_sb[:], xr1_gs[slot * P:(slot + 1) * P, :])
            pm = pse.tile([P, HC3], F32, space="PSUM", tag="pm")
            edge_slot(ci, int(s_sched[slot]), gs_idx_t, gs_dstl_t, tbl_g,
                      xr_sb, att1_gs_t, HC1, pm, None)
            ci += int(s_sched[slot])
            y = norm_heads(pm[:, :HC1], pm[:, HC1:HC1 + H], HC1, "y1s")
            y2 = ev.tile([P, HC1], F32, tag="y2s")
            nc.vector.tensor_tensor(out=y2[:], in0=y[:], in1=bias1_gs_t[:],
                                    op=OP.add)
            y3 = ev.tile([P, HC1], F32, tag="y3s")
            nc.vector.tensor_tensor(
                out=_ap(y3, 0, [[C1, H], [1, C1]]),
                in0=_ap(y2, 0, [[C1, H], [1, C1]]),
                in1=_ap(sl1_sb, slot * C1, [[0, H], [1, C1]]),
                op=OP.add)
            x1 = ev.tile([P, HC1], F32, tag="x1s")
            _elu(x1[:], y3[:], HC1)
            lhsT = x1_transposed(x1[:], "x1sT")
            dense_T(lhsT, Wr3_t, HC3, br3_t,
                    dst_ap=xr3[slot * P:(slot + 1) * P, :])
            dense_T(lhsT, sl3_W_t, C3, sl3_b_t,
                    sbuf_dst=sl3_sb[:, slot * C3:(slot + 1) * C3])

        # ---- phase D: gs edges layer 3 -> output
        ci = 0
        for slot in range(S_PER_CORE):
            xr_sb = sb.tile([P, HC3], BF16, tag="xr3")
            nc.sync.dma_start(xr_sb[:], xr3[slot * P:(slot + 1) * P, :])
            pm = pse.tile([P, HC3], F32, space="PSUM", tag="pm")
            pden = ps1.tile([P, H], F32, space="PSUM", tag="pden")
            edge_slot(ci, int(s_sched[slot]), gs_idx_t, gs_dstl_t, tbl_3,
                      xr_sb, att3_t, HC3, pm, pden)
            ci += int(s_sched[slot])
            den4 = ev.tile([P, H], F32, tag="den")
            nc.vector.tensor_scalar(out=den4[:], in0=pden[:], scalar1=4.0,
                                    scalar2=4e-16, op0=OP.mult, op1=OP.add)
            rden = ev.tile([P, H], F32, tag="rden")
            nc.vector.reciprocal(rden[:], den4[:])
            base = ev.tile([P, C3], F32, tag="based")
            nc.vector.tensor_tensor(out=base[:],
                                    in0=sl3_sb[:, slot * C3:(slot + 1) * C3],
                                    in1=bias3_t[:], op=OP.add)
            accs = [base]
            for h in range(H):
                a = ev.tile([P, C3], F32, tag=f"acc{h}")
                nc.vector.scalar_tensor_tensor(
                    out=a[:], in0=pm[:, h * C3:(h + 1) * C3],
                    scalar=rden[:, h:h + 1], in1=accs[-1][:],
                    op0=OP.mult, op1=OP.add)
                accs.append(a)
            o = ev.tile([P, C3], F32, tag="outt")
            _elu(o[:], accs[-1][:], C3)
            nc.sync.dma_start(out_own[slot * P:(slot + 1) * P, :], o[:])

    nc.compile()
    return nc


# ------------------------------------------------------------------ driver

_CACHE = {}


def kernel(**inputs):
    plan, in_maps = _plan(inputs)
    key = (tuple(plan["g_sched"]), tuple(plan["s_sched"]))
    if key not in _CACHE:
        _CACHE[key] = _build(plan["g_sched"], plan["s_sched"])
    nc = _CACHE[key]
    r = run_bass_kernel_spmd(nc, in_maps, core_ids=list(range(NCORES)))
    out = np.zeros((NS, C3), np.float32)
    s_assign = plan["s_assign"]
    for c in range(NCORES):
        oc = r.results[c]["out_own"]
        for slot in range(S_PER_CORE):
            t = s_assign[slot, c]
            out[t * P:(t + 1) * P] = oc[slot * P:(slot + 1) * P]
    return out


# revision 18
# speedup vs baseline: 1.0823x; 1.0637x over previous
"""BiRGAT (bipartite 2-layer GATv2) Trainium2 kernel, 8-core SPMD.

Strategy: destination-tile sharding (as before), with a bf16 batched edge
pipeline. Gene dsts padded to 160 tiles of 128 (20/core), sample dsts 32
tiles (4/core), dealt to cores by sorted chunk count. Source-side node
tables are bf16 in DRAM, AllGathered between phases.

Edge phase per slot, in groups of <=8 128-edge chunks:
- ONE batched indirect gather of source rows [128, G, w] (amortizes the
  ~1us SWDGE descriptor-gen cost on the Pool engine).
- one-hot per chunk via a single TensorScalarPtr (bf16, 4x DVE mode).
- PE transposes -> batched PSUM->SBUF copy gives ohT (bf16).
- z = ident@xl + ohT@xr on PE (PSUM fp32), Prelu on Act (2-chunk batch).
- alpha: batched (g * att_b) TT (bf16 2x), log2 fold adds 64->8, one
  tensor_reduce to [128, G*H] fp32, batched Exp.
- messages: layer1 = xl * ea broadcast (one TT per group) + scatter
  matmul [w+H] (ea cols give the denominator); layer3 = per-head
  ea-scaled one-hots (TSPtr is_equal*mult) + per-head scatter matmuls.
Tile evacuation does softmax normalization (no segment_max; alpha std
~0.3 so den+1e-16 makes max-subtraction irrelevant), bias, ELU, residuals.
"""
import sys

sys.path.insert(0, "/opt/trn_rl_repo")

import numpy as np
import ml_dtypes
from contextlib import ExitStack

import concourse.bass as bass
import concourse.tile as tile
from concourse import bacc, mybir
from concourse.bass_utils import run_bass_kernel_spmd
from concourse.masks import make_identity

P = 128
NCORES = 8
NS, NG, E = 4096, 20000, 131072
DIN, H, C1, C3 = 256, 4, 64, 128
HC1, HC3 = H * C1, H * C3          # 256, 512
NGP = 20480                        # genes padded to 160 tiles
NST, NGT = NS // P, NGP // P       # 32, 160
S_PER_CORE, G_PER_CORE = NST // NCORES, NGT // NCORES   # 4, 20
SROWS, GROWS = S_PER_CORE * P, G_PER_CORE * P           # 512, 2560

F32 = mybir.dt.float32
BF16 = mybir.dt.bfloat16
I32 = mybir.dt.int32
I16 = mybir.dt.int16
AF = mybir.ActivationFunctionType
OP = mybir.AluOpType
AX = mybir.AxisListType
BF = ml_dtypes.bfloat16

PAD_LOC = 200.0   # dst-local sentinel for padded edges (never equals 0..127)
GMAX = 8          # chunks per gather/DVE batch group
import os
SIM_SAFE = os.environ.get("BIRGAT_SIM_SAFE", "0") == "1"  # CoreSim lacks Prelu


# ---------------------------------------------------------------- host plan

def _deal_tiles(dst, n_tiles, per_core):
    tcnt = np.bincount(dst // P, minlength=n_tiles)
    chunks = np.maximum((tcnt + P - 1) // P, 1)
    order = np.argsort(-chunks, kind="stable")
    assign = order.reshape(per_core, NCORES)
    sched = chunks[assign].max(axis=1)
    return assign, sched.astype(int), tcnt


def _edge_arrays(src, dst, assign, sched, src_row_map, core):
    """Per-core edge chunk arrays for one relation.
    Returns src_rows [P, NCH] i32, dstloc [P, NCH] bf16."""
    nch = int(sched.sum())
    src_rows = np.zeros((nch, P), np.int32)
    dstloc = np.full((nch, P), PAD_LOC, np.float32)
    tile_of = dst // P
    ci = 0
    for slot in range(len(sched)):
        t = assign[slot, core]
        e = np.nonzero(tile_of == t)[0]
        n = len(e)
        want = sched[slot] * P
        s = np.zeros(want, np.int32)
        dl = np.full(want, PAD_LOC, np.float32)
        s[:n] = src_row_map[src[e]]
        dl[:n] = (dst[e] % P).astype(np.float32)
        src_rows[ci:ci + sched[slot]] = s.reshape(-1, P)
        dstloc[ci:ci + sched[slot]] = dl.reshape(-1, P)
        ci += sched[slot]
    idx16 = np.tile(src_rows.reshape(-1, 16).T.astype(np.int16), (8, 1))
    return idx16.copy(), dstloc.T.copy()


def _bcast(v, dt=np.float32, p=P):
    return np.broadcast_to(np.asarray(v, np.float32).reshape(1, -1),
                           (p, len(np.asarray(v).reshape(-1)))).astype(dt)


def _wtile(w, kdim, n):
    """Host-side [128, kdim//128, n] bf16 rhs layout for matmul weights."""
    a = np.asarray(w, np.float32).reshape(kdim // P, P, n)
    return a.transpose(1, 0, 2).astype(BF).copy()


def _xT(x, rows):
    """Host-side [128, DIN//128, rows] bf16 transposed-feature staging."""
    a = np.zeros((DIN, rows), np.float32)
    a[:, :x.shape[0]] = np.asarray(x, np.float32).T
    return a.reshape(DIN // P, P, rows).transpose(1, 0, 2).astype(BF).copy()


def _plan(inputs):
    sg_src = np.asarray(inputs["sg_src"]); sg_dst = np.asarray(inputs["sg_dst"])
    gs_src = np.asarray(inputs["gs_src"]); gs_dst = np.asarray(inputs["gs_dst"])

    g_assign, g_sched, _ = _deal_tiles(sg_dst, NGT, G_PER_CORE)
    s_assign, s_sched, _ = _deal_tiles(gs_dst, NST, S_PER_CORE)

    g_owner = np.zeros(NGT, np.int32); g_slot = np.zeros(NGT, np.int32)
    for slot in range(G_PER_CORE):
        for c in range(NCORES):
            g_owner[g_assign[slot, c]] = c
            g_slot[g_assign[slot, c]] = slot
    s_owner = np.zeros(NST, np.int32); s_slot = np.zeros(NST, np.int32)
    for slot in range(S_PER_CORE):
        for c in range(NCORES):
            s_owner[s_assign[slot, c]] = c
            s_slot[s_assign[slot, c]] = slot

    sid = np.arange(NS)
    srow_tbl = s_owner[sid // P] * SROWS + s_slot[sid // P] * P + sid % P
    gid = np.arange(NG)
    grow_tbl = g_owner[gid // P] * GROWS + g_slot[gid // P] * P + gid % P

    plan = {
        "g_assign": g_assign, "g_sched": g_sched,
        "s_assign": s_assign, "s_sched": s_sched,
    }

    x_sample = np.asarray(inputs["x_sample"], np.float32)
    x_gene = np.asarray(inputs["x_gene"], np.float32)

    in_maps = []
    for c in range(NCORES):
        s_tiles = s_assign[:, c]
        xs_own = x_sample.reshape(NST, P, DIN)[s_tiles].reshape(SROWS, DIN)
        g_tiles = g_assign[:, c]
        xg_own = np.zeros((GROWS, DIN), np.float32)
        for i, t in enumerate(g_tiles):
            lo = t * P
            if lo < NG:
                n = min(P, NG - lo)
                xg_own[i * P:i * P + n] = x_gene[lo:lo + n]

        sgS, sgL = _edge_arrays(sg_src, sg_dst, g_assign, g_sched, srow_tbl, c)
        gsS, gsL = _edge_arrays(gs_src, gs_dst, s_assign, s_sched, grow_tbl, c)

        m = {
            "xsT_own": _xT(xs_own, SROWS),
            "xgT_own": _xT(xg_own, GROWS),
            "Wl1_sg": _wtile(inputs["Wl1_sg"], DIN, HC1),
            "Wr1_sg": _wtile(inputs["Wr1_sg"], DIN, HC1),
            "Wl1_gs": _wtile(inputs["Wl1_gs"], DIN, HC1),
            "Wr1_gs": _wtile(inputs["Wr1_gs"], DIN, HC1),
            "Wl3": _wtile(inputs["Wl3_gs"], HC1, HC3),
            "Wr3": _wtile(inputs["Wr3_gs"], HC1, HC3),
            "sl1_W": _wtile(inputs["sl1_W"], DIN, C1),
            "sl3_W": _wtile(inputs["sl3_W"], HC1, C3),
            "att1_sg_b": _bcast(np.asarray(inputs["att1_sg"]).reshape(-1), BF),
            "att1_gs_b": _bcast(np.asarray(inputs["att1_gs"]).reshape(-1), BF),
            "att3_b": _bcast(np.asarray(inputs["att3_gs"]).reshape(-1), BF),
            "bl1_sg_b": _bcast(inputs["bl1_sg"]),
            "br1_sg_b": _bcast(inputs["br1_sg"]),
            "bl1_gs_b": _bcast(inputs["bl1_gs"]),
            "br1_gs_b": _bcast(inputs["br1_gs"]),
            "bias1_sg_b": _bcast(inputs["bias1_sg"]),
            "bias1_gs_b": _bcast(inputs["bias1_gs"]),
            "bl3_b": _bcast(inputs["bl3_gs"]),
            "br3_b": _bcast(inputs["br3_gs"]),
            "bias3_b": _bcast(inputs["bias3_gs"]),
            "sl1_b_b": _bcast(inputs["sl1_b"]),
            "sl3_b_b": _bcast(inputs["sl3_b"]),
            "sg_idx": sgS, "sg_dstl": sgL,
            "gs_idx": gsS, "gs_dstl": gsL,
        }
        in_maps.append(m)
    return plan, in_maps


# ------------------------------------------------------------- device build

def _groups(nch):
    out = []
    ci = 0
    while ci < nch:
        g = min(GMAX, nch - ci)
        out.append((ci, g))
        ci += g
    return out


def _ap(base, offset_cols, shape_strides):
    """AP over base tile's tensor: shape_strides = [[stride, n], ...] free."""
    a = base[:]
    return bass.AP(a.tensor, a.offset + offset_cols,
                   [[a.ap[0][0], P]] + shape_strides)


def _build(g_sched, s_sched):
    nsg = int(g_sched.sum())
    ngs = int(s_sched.sum())
    nc = bacc.Bacc("TRN2", target_bir_lowering=False, debug=False,
                   num_devices=NCORES)

    ei = lambda name, shape, dt=F32: nc.dram_tensor(name, shape, dt,
                                                    kind="ExternalInput")
    xsT_own = ei("xsT_own", [P, DIN // P, SROWS], BF16)
    xgT_own = ei("xgT_own", [P, DIN // P, GROWS], BF16)
    Wl1_sg = ei("Wl1_sg", [P, 2, HC1], BF16); Wr1_sg = ei("Wr1_sg", [P, 2, HC1], BF16)
    Wl1_gs = ei("Wl1_gs", [P, 2, HC1], BF16); Wr1_gs = ei("Wr1_gs", [P, 2, HC1], BF16)
    Wl3 = ei("Wl3", [P, 2, HC3], BF16); Wr3 = ei("Wr3", [P, 2, HC3], BF16)
    sl1_W = ei("sl1_W", [P, 2, C1], BF16); sl3_W = ei("sl3_W", [P, 2, C3], BF16)
    att1_sg_b = ei("att1_sg_b", [P, HC1], BF16)
    att1_gs_b = ei("att1_gs_b", [P, HC1], BF16)
    att3_b = ei("att3_b", [P, HC3], BF16)
    bl1_sg_b = ei("bl1_sg_b", [P, HC1]); br1_sg_b = ei("br1_sg_b", [P, HC1])
    bl1_gs_b = ei("bl1_gs_b", [P, HC1]); br1_gs_b = ei("br1_gs_b", [P, HC1])
    bias1_sg_b = ei("bias1_sg_b", [P, HC1]); bias1_gs_b = ei("bias1_gs_b", [P, HC1])
    bl3_b = ei("bl3_b", [P, HC3]); br3_b = ei("br3_b", [P, HC3])
    bias3_b = ei("bias3_b", [P, C3])
    sl1_b_b = ei("sl1_b_b", [P, C1]); sl3_b_b = ei("sl3_b_b", [P, C3])
    sg_idx = ei("sg_idx", [P, nsg * 8], I16)
    sg_dstl = ei("sg_dstl", [P, nsg])
    gs_idx = ei("gs_idx", [P, ngs * 8], I16)
    gs_dstl = ei("gs_dstl", [P, ngs])

    out_own = nc.dram_tensor("out_own", [SROWS, C3], F32, kind="ExternalOutput")

    # DRAM scratch (tables bf16)
    agin_s = nc.dram_tensor("agin_s", [SROWS, HC1], BF16)
    agin_g = nc.dram_tensor("agin_g", [GROWS, HC1], BF16)
    agin_3 = nc.dram_tensor("agin_3", [GROWS, HC3], BF16)
    tbl_s = nc.dram_tensor("tbl_s", [NS, HC1], BF16, addr_space="Shared")
    tbl_g = nc.dram_tensor("tbl_g", [NGP, HC1], BF16, addr_space="Shared")
    tbl_3 = nc.dram_tensor("tbl_3", [NGP, HC3], BF16, addr_space="Shared")
    xr1_sg = nc.dram_tensor("xr1_sg", [GROWS, HC1], BF16)
    xr1_gs = nc.dram_tensor("xr1_gs", [SROWS, HC1], BF16)
    xr3 = nc.dram_tensor("xr3", [SROWS, HC3], BF16)

    RG = [list(range(NCORES))]

    with tile.TileContext(nc) as tc, ExitStack() as ctx:
        res = ctx.enter_context(tc.tile_pool(name="res", bufs=1))
        se = ctx.enter_context(tc.tile_pool(name="se", bufs=3))
        sx1 = ctx.enter_context(tc.tile_pool(name="sx1", bufs=3))
        sx3 = ctx.enter_context(tc.tile_pool(name="sx3", bufs=2))
        ev = ctx.enter_context(tc.tile_pool(name="ev", bufs=2))
        psz = ctx.enter_context(tc.tile_pool(name="psz", bufs=2, space="PSUM"))
        psoh = ctx.enter_context(tc.tile_pool(name="psoh", bufs=1, space="PSUM"))
        pse = ctx.enter_context(tc.tile_pool(name="pse", bufs=2, space="PSUM"))
        ps1 = ctx.enter_context(tc.tile_pool(name="ps1", bufs=2, space="PSUM"))
        psd = ctx.enter_context(tc.tile_pool(name="psd", bufs=1, space="PSUM"))

        ident = res.tile([P, P], F32)
        make_identity(nc, ident[:])
        ident_bf = res.tile([P, P], BF16)
        nc.scalar.copy(ident_bf[:], ident[:])
        iota_bf = res.tile([P, P], BF16)
        nc.gpsimd.iota(iota_bf[:], pattern=[[1, P]], base=0,
                       channel_multiplier=0,
                       allow_small_or_imprecise_dtypes=True)
        ones_bf = res.tile([P, 1], BF16)
        nc.vector.memset(ones_bf[:], 1.0)

        def rload(name, dram, shape, dt=F32):
            t = res.tile(shape, dt, tag=name)
            nc.sync.dma_start(t[:], dram[:])
            return t

        att1_sg_t = rload("a1s", att1_sg_b, [P, HC1], BF16)
        att1_gs_t = rload("a1g", att1_gs_b, [P, HC1], BF16)
        att3_t = rload("a3", att3_b, [P, HC3], BF16)
        bias1_sg_t = rload("b1s", bias1_sg_b, [P, HC1])
        bias1_gs_t = rload("b1g", bias1_gs_b, [P, HC1])
        bias3_t = rload("b3", bias3_b, [P, C3])
        sg_idx_t = rload("sgs", sg_idx, [P, nsg * 8], I16)
        sg_dstl_t = rload("sgl", sg_dstl, [P, nsg])
        gs_idx_t = rload("gss", gs_idx, [P, ngs * 8], I16)
        gs_dstl_t = rload("gsl", gs_dstl, [P, ngs])
        xsT = rload("xsT", xsT_own, [P, DIN // P, SROWS], BF16)
        xgT = rload("xgT", xgT_own, [P, DIN // P, GROWS], BF16)

        Wl1_sg_t = rload("Wl1_sg", Wl1_sg, [P, 2, HC1], BF16)
        Wr1_sg_t = rload("Wr1_sg", Wr1_sg, [P, 2, HC1], BF16)
        Wl1_gs_t = rload("Wl1_gs", Wl1_gs, [P, 2, HC1], BF16)
        Wr1_gs_t = rload("Wr1_gs", Wr1_gs, [P, 2, HC1], BF16)
        Wl3_t = rload("Wl3", Wl3, [P, 2, HC3], BF16)
        Wr3_t = rload("Wr3", Wr3, [P, 2, HC3], BF16)
        sl1_W_t = rload("sl1_W", sl1_W, [P, 2, C1], BF16)
        sl3_W_t = rload("sl3_W", sl3_W, [P, 2, C3], BF16)
        bl1_sg_t = rload("bl1s", bl1_sg_b, [P, HC1])
        br1_sg_t = rload("br1s", br1_sg_b, [P, HC1])
        bl1_gs_t = rload("bl1g", bl1_gs_b, [P, HC1])
        br1_gs_t = rload("br1g", br1_gs_b, [P, HC1])
        bl3_t = rload("bl3", bl3_b, [P, HC3])
        br3_t = rload("br3", br3_b, [P, HC3])
        sl1_b_t = rload("sl1b", sl1_b_b, [P, C1])
        sl3_b_t = rload("sl3b", sl3_b_b, [P, C3])

        sl1_sb = res.tile([P, S_PER_CORE * C1], F32)
        sl3_sb = res.tile([P, S_PER_CORE * C3], F32)

        def dense_T(lhsT, w_sb, n, bias_t, dst_ap=None, sbuf_dst=None,
                    rdt=BF16, tag="dout"):
            """out = lhsT.T @ W + bias; lhsT = list of [128,128] bf16 APs."""
            pt = ps1.tile([P, HC3], F32, space="PSUM", tag="aux")
            for k in range(len(lhsT)):
                nc.tensor.matmul(pt[:, :n], lhsT=lhsT[k], rhs=w_sb[:, k, :n],
                                 start=(k == 0), stop=(k == len(lhsT) - 1))
            o = sbuf_dst
            if o is None:
                ot = se.tile([P, n], rdt, tag=tag)
                o = ot[:]
            nc.vector.tensor_tensor(out=o, in0=pt[:, :n], in1=bias_t[:, :n],
                                    op=OP.add)
            if dst_ap is not None:
                nc.sync.dma_start(dst_ap, ot[:])
            return o

        # ---- phase A: node tables (no on-chip transposes; xT staged)
        for i in range(S_PER_CORE):
            lhsT = [xsT[:, k, i * P:(i + 1) * P] for k in range(DIN // P)]
            dense_T(lhsT, Wl1_sg_t, HC1, bl1_sg_t,
                    dst_ap=agin_s[i * P:(i + 1) * P, :])
            dense_T(lhsT, Wr1_gs_t, HC1, br1_gs_t,
                    dst_ap=xr1_gs[i * P:(i + 1) * P, :])
            dense_T(lhsT, sl1_W_t, C1, sl1_b_t,
                    sbuf_dst=sl1_sb[:, i * C1:(i + 1) * C1])
        nc.gpsimd.collective_compute("AllGather", OP.bypass, replica_groups=RG,
                                     ins=[agin_s[:]], outs=[tbl_s[:]])

        for j in range(G_PER_CORE):
            lhsT = [xgT[:, k, j * P:(j + 1) * P] for k in range(DIN // P)]
            dense_T(lhsT, Wl1_gs_t, HC1, bl1_gs_t,
                    dst_ap=agin_g[j * P:(j + 1) * P, :])
            dense_T(lhsT, Wr1_sg_t, HC1, br1_sg_t,
                    dst_ap=xr1_sg[j * P:(j + 1) * P, :])
        nc.gpsimd.collective_compute("AllGather", OP.bypass, replica_groups=RG,
                                     ins=[agin_g[:]], outs=[tbl_g[:]])

        # ---- edge slot processing
        def edge_slot(ci0, nch, idx_t, dstl_t, tbl, xr_sb, att_t, w, pm, pden):
            """Process one dst slot's nch chunks; accumulate into pm (+pden
            for layer3 mode, which is signalled by pden is not None)."""
            l3 = pden is not None
            sx = sx3 if l3 else sx1
            nfold = 4 if w == HC3 else 3   # per-head 128->8 or 64->8
            ch = w // H
            for gi, (goff, G) in enumerate(_groups(nch)):
                ci = ci0 + goff
                first0 = goff == 0
                # batched gather [128, G, w]
                xl = sx.tile([P, GMAX, w], BF16, tag=f"xl{w}")
                nc.gpsimd.dma_gather(
                    out_ap=_ap(xl, 0, [[w, G], [1, w]]), in_ap=tbl[:],
                    idxs_ap=idx_t[:, ci * 8:(ci + G) * 8],
                    num_idxs=G * P, num_idxs_reg=G * P, elem_size=w)
                # one-hots (bf16, 4x TSPtr) + PE transposes -> ohT bf16
                onehot = se.tile([P, GMAX, P], BF16, tag="onehot")
                ohT = se.tile([P, GMAX, P], BF16, tag="ohT")
                for g in range(G):
                    nc.vector.tensor_scalar(
                        out=onehot[:, g, :], in0=iota_bf[:],
                        scalar1=dstl_t[:, ci + g:ci + g + 1], scalar2=None,
                        op0=OP.is_equal)
                pt = psoh.tile([P, GMAX, P], BF16, space="PSUM", tag="ohTp")
                for g in range(G):
                    nc.tensor.transpose(out=pt[:, g, :],
                                        in_=onehot[:, g, :],
                                        identity=ident_bf[:])
                nc.scalar.activation(
                    _ap(ohT, 0, [[P, G], [1, P]]),
                    _ap(pt, 0, [[P, G], [1, P]]), AF.Copy)
                # z = ident@xl + ohT@xr (PSUM fp32), Prelu -> gt bf16
                # (gt doubles as the msgs tile: [xl*ea | ea] after exp/mult)
                wh = w + H
                gt = sx.tile([P, GMAX, wh], BF16, tag=f"g{w}")
                gm = sx.tile([P, GMAX, w], BF16, tag=f"gm{w}")
                zb = 1 if l3 else 2   # chunks per PSUM z tile (2KB bank)
                for z0 in range(0, G, zb):
                    zn = min(zb, G - z0)
                    zp = psz.tile([P, HC3], F32, space="PSUM", tag="z")
                    for g in range(z0, z0 + zn):
                        zv = _ap(zp, (g - z0) * w, [[1, w]])
                        nc.tensor.matmul(zv, lhsT=ident_bf[:],
                                         rhs=xl[:, g, :], start=True,
                                         stop=False)
                        nc.tensor.matmul(zv, lhsT=ohT[:, g, :],
                                         rhs=xr_sb[:, :w], start=False,
                                         stop=True)
                    if SIM_SAFE:
                        nc.vector.scalar_tensor_tensor(
                            out=_ap(gt, z0 * wh, [[wh, zn], [1, w]]),
                            in0=_ap(zp, 0, [[w, zn], [1, w]]), scalar=0.2,
                            in1=_ap(zp, 0, [[w, zn], [1, w]]),
                            op0=OP.mult, op1=OP.max)
                    else:
                        nc.scalar.activation(
                            _ap(gt, z0 * wh, [[wh, zn], [1, w]]),
                            _ap(zp, 0, [[w, zn], [1, w]]), AF.Prelu, alpha=0.2)
                    # alpha stage 1: gm = g*att (bf16 2x), per z-pair
                    nc.vector.tensor_tensor(
                        out=_ap(gm, z0 * w, [[w, zn], [1, w]]),
                        in0=_ap(gt, z0 * wh, [[wh, zn], [1, w]]),
                        in1=_ap(att_t, 0, [[0, zn], [1, w]]), op=OP.mult)
                src_t, src_w, src_off = gm, w, 0
                for f in range(nfold):
                    hw_ = src_w // H // 2       # half of per-head width
                    ft = sx.tile([P, GMAX * H * hw_], BF16, tag=f"f{w}_{f}")
                    nc.vector.tensor_tensor(
                        out=_ap(ft, 0, [[hw_, G * H], [1, hw_]]),
                        in0=_ap(src_t, src_off, [[2 * hw_, G * H], [1, hw_]]),
                        in1=_ap(src_t, src_off + hw_,
                                [[2 * hw_, G * H], [1, hw_]]),
                        op=OP.add)
                    src_t, src_w, src_off = ft, src_w // 2, 0
                alpha = se.tile([P, GMAX * H], F32, tag="alpha")
                nc.vector.tensor_reduce(
                    out=_ap(alpha, 0, [[1, G * H]]),
                    in_=_ap(src_t, 0, [[8, G * H], [1, 8]]),
                    axis=AX.X, op=OP.add)
                nc.scalar.activation(
                    _ap(gt, w, [[wh, G], [1, H]]),
                    _ap(alpha, 0, [[H, G], [1, H]]), AF.Exp)
                nc.vector.tensor_tensor(
                    out=_ap(gt, 0, [[wh, G], [ch, H], [1, ch]]),
                    in0=_ap(xl, 0, [[w, G], [ch, H], [1, ch]]),
                    in1=_ap(gt, w, [[wh, G], [1, H], [0, ch]]),
                    op=OP.mult)
                for g in range(G):
                    st = first0 and g == 0
                    sp = goff + g == nch - 1
                    if l3:
                        nc.tensor.matmul(pm[:, :w], lhsT=onehot[:, g, :],
                                         rhs=gt[:, g, :w], start=st, stop=sp)
                        nc.tensor.matmul(pden[:, :H], lhsT=onehot[:, g, :],
                                         rhs=gt[:, g, w:w + H],
                                         start=st, stop=sp)
                    else:
                        nc.tensor.matmul(pm[:, :w + H], lhsT=onehot[:, g, :],
                                         rhs=gt[:, g, :], start=st, stop=sp)

        def norm_heads(psum_ap, den_ap, w, tag):
            ch = w // H
            den = ev.tile([P, H], F32, tag="den")
            nc.vector.tensor_scalar(out=den[:], in0=den_ap, scalar1=1e-16,
                                    scalar2=None, op0=OP.add)
            rden = ev.tile([P, H], F32, tag="rden")
            nc.vector.reciprocal(rden[:], den[:])
            y = ev.tile([P, w], F32, tag=tag)
            nc.vector.tensor_tensor(
                out=_ap(y, 0, [[ch, H], [1, ch]]),
                in0=psum_ap,
                in1=_ap(rden, 0, [[1, H], [0, ch]]),
                op=OP.mult)
            return y

        def _elu(out_ap, y_ap, w):
            m = ev.tile([P, w], F32, tag="elu_m")
            nc.vector.tensor_scalar(out=m[:], in0=y_ap, scalar1=0.0,
                                    scalar2=None, op0=OP.min)
            e = ev.tile([P, w], F32, tag="elu_e")
            nc.scalar.activation(e[:], m[:], AF.Exp)
            r = ev.tile([P, w], F32, tag="elu_r")
            nc.scalar.activation(r[:], y_ap, AF.Relu)
            nc.vector.scalar_tensor_tensor(out=out_ap, in0=r[:], scalar=-1.0,
                                           in1=e[:], op0=OP.add, op1=OP.add)

        def x1_transposed(x1_ap, tag):
            """fp32 x1 [128, HC1] -> bf16, PE-transpose -> 2 bf16 lhsT."""
            xb = ev.tile([P, HC1], BF16, tag=tag + "b")
            nc.scalar.activation(xb[:], x1_ap, AF.Copy)
            pt = psoh.tile([P, GMAX, P], BF16, space="PSUM", tag="ohTp")
            for k in range(2):
                nc.tensor.transpose(out=pt[:, k, :], in_=xb[:, k * P:(k + 1) * P],
                                    identity=ident_bf[:])
            xT2 = ev.tile([P, 2, P], BF16, tag=tag + "T")
            nc.scalar.activation(_ap(xT2, 0, [[P, 2], [1, P]]),
                                 _ap(pt, 0, [[P, 2], [1, P]]), AF.Copy)
            return [xT2[:, 0, :], xT2[:, 1, :]]

        # ---- phase B: sg edges -> x1_gene -> xl3 rows, AG2
        ci = 0
        for slot in range(G_PER_CORE):
            xr_sb = se.tile([P, HC1], BF16, tag="xr1")
            nc.sync.dma_start(xr_sb[:], xr1_sg[slot * P:(slot + 1) * P, :])
            pm = pse.tile([P, HC3], F32, space="PSUM", tag="pm")
            edge_slot(ci, int(g_sched[slot]), sg_idx_t, sg_dstl_t, tbl_s,
                      xr_sb, att1_sg_t, HC1, pm, None)
            ci += int(g_sched[slot])
            y = norm_heads(pm[:, :HC1], pm[:, HC1:HC1 + H], HC1, "y1g")
            y2 = ev.tile([P, HC1], F32, tag="y2g")
            nc.vector.tensor_tensor(out=y2[:], in0=y[:], in1=bias1_sg_t[:],
                                    op=OP.add)
            x1 = ev.tile([P, HC1], F32, tag="x1g")
            _elu(x1[:], y2[:], HC1)
            lhsT = x1_transposed(x1[:], "x1gT")
            dense_T(lhsT, Wl3_t, HC3, bl3_t,
                    dst_ap=agin_3[slot * P:(slot + 1) * P, :])
        nc.gpsimd.collective_compute("AllGather", OP.bypass, replica_groups=RG,
                                     ins=[agin_3[:]], outs=[tbl_3[:]])

        # ---- phase C: gs edges -> x1_sample -> xr3/sl3 rows
        ci = 0
        for slot in range(S_PER_CORE):
            xr_sb = se.tile([P, HC1], BF16, tag="xr1")
            nc.sync.dma_start(xr_sb[:], xr1_gs[slot * P:(slot + 1) * P, :])
            pm = pse.tile([P, HC3], F32, space="PSUM", tag="pm")
            edge_slot(ci, int(s_sched[slot]), gs_idx_t, gs_dstl_t, tbl_g,
                      xr_sb, att1_gs_t, HC1, pm, None)
            ci += int(s_sched[slot])
            y = norm_heads(pm[:, :HC1], pm[:, HC1:HC1 + H], HC1, "y1s")
            y2 = ev.tile([P, HC1], F32, tag="y2s")
            nc.vector.tensor_tensor(out=y2[:], in0=y[:], in1=bias1_gs_t[:],
                                    op=OP.add)
            y3 = ev.tile([P, HC1], F32, tag="y3s")
            nc.vector.tensor_tensor(
                out=_ap(y3, 0, [[C1, H], [1, C1]]),
                in0=_ap(y2, 0, [[C1, H], [1, C1]]),
                in1=_ap(sl1_sb, slot * C1, [[0, H], [1, C1]]),
                op=OP.add)
            x1 = ev.tile([P, HC1], F32, tag="x1s")
            _elu(x1[:], y3[:], HC1)
            lhsT = x1_transposed(x1[:], "x1sT")
            dense_T(lhsT, Wr3_t, HC3, br3_t,
                    dst_ap=xr3[slot * P:(slot + 1) * P, :])
            dense_T(lhsT, sl3_W_t, C3, sl3_b_t,
                    sbuf_dst=sl3_sb[:, slot * C3:(slot + 1) * C3])

        # ---- phase D: gs edges layer 3 -> output
        ci = 0
        for slot in range(S_PER_CORE):
            xr_sb = se.tile([P, HC3], BF16, tag="xr3")
            nc.sync.dma_start(xr_sb[:], xr3[slot * P:(slot + 1) * P, :])
            pm = pse.tile([P, HC3], F32, space="PSUM", tag="pm")
            pden = psd.tile([P, H], F32, space="PSUM", tag="pden")
            edge_slot(ci, int(s_sched[slot]), gs_idx_t, gs_dstl_t, tbl_3,
                      xr_sb, att3_t, HC3, pm, pden)
            ci += int(s_sched[slot])
            den4 = ev.tile([P, H], F32, tag="den")
            nc.vector.tensor_scalar(out=den4[:], in0=pden[:], scalar1=4.0,
                                    scalar2=4e-16, op0=OP.mult, op1=OP.add)
            rden = ev.tile([P, H], F32, tag="rden")
            nc.vector.reciprocal(rden[:], den4[:])
            base = ev.tile([P, C3], F32, tag="based")
            nc.vector.tensor_tensor(out=base[:],
                                    in0=sl3_sb[:, slot * C3:(slot + 1) * C3],
                                    in1=bias3_t[:], op=OP.add)
            accs = [base]
            for h in range(H):
                a = ev.tile([P, C3], F32, tag=f"acc{h}")
                nc.vector.scalar_tensor_tensor(
                    out=a[:], in0=pm[:, h * C3:(h + 1) * C3],
                    scalar=rden[:, h:h + 1], in1=accs[-1][:],
                    op0=OP.mult, op1=OP.add)
                accs.append(a)
            o = ev.tile([P, C3], F32, tag="outt")
            _elu(o[:], accs[-1][:], C3)
            nc.sync.dma_start(out_own[slot * P:(slot + 1) * P, :], o[:])

    nc.compile()
    return nc


# ------------------------------------------------------------------ driver

_CACHE = {}


def kernel(**inputs):
    plan, in_maps = _plan(inputs)
    key = (tuple(plan["g_sched"]), tuple(plan["s_sched"]))
    if key not in _CACHE:
        _CACHE[key] = _build(plan["g_sched"], plan["s_sched"])
    nc = _CACHE[key]
    r = run_bass_kernel_spmd(nc, in_maps, core_ids=list(range(NCORES)))
    out = np.zeros((NS, C3), np.float32)
    s_assign = plan["s_assign"]
    for c in range(NCORES):
        oc = r.results[c]["out_own"]
        for slot in range(S_PER_CORE):
            t = s_assign[slot, c]
            out[t * P:(t + 1) * P] = oc[slot * P:(slot + 1) * P]
    return out


# revision 20
# speedup vs baseline: 1.2106x; 1.1185x over previous
"""BiRGAT (bipartite 2-layer GATv2) Trainium2 kernel, 8-core SPMD.

Strategy: destination-tile sharding (as before), with a bf16 batched edge
pipeline. Gene dsts padded to 160 tiles of 128 (20/core), sample dsts 32
tiles (4/core), dealt to cores by sorted chunk count. Source-side node
tables are bf16 in DRAM, AllGathered between phases.

Edge phase per slot, in groups of <=8 128-edge chunks:
- ONE batched indirect gather of source rows [128, G, w] (amortizes the
  ~1us SWDGE descriptor-gen cost on the Pool engine).
- one-hot per chunk via a single TensorScalarPtr (bf16, 4x DVE mode).
- PE transposes -> batched PSUM->SBUF copy gives ohT (bf16).
- z = ident@xl + ohT@xr on PE (PSUM fp32), Prelu on Act (2-chunk batch).
- alpha: batched (g * att_b) TT (bf16 2x), log2 fold adds 64->8, one
  tensor_reduce to [128, G*H] fp32, batched Exp.
- messages: layer1 = xl * ea broadcast (one TT per group) + scatter
  matmul [w+H] (ea cols give the denominator); layer3 = per-head
  ea-scaled one-hots (TSPtr is_equal*mult) + per-head scatter matmuls.
Tile evacuation does softmax normalization (no segment_max; alpha std
~0.3 so den+1e-16 makes max-subtraction irrelevant), bias, ELU, residuals.
"""
import sys

sys.path.insert(0, "/opt/trn_rl_repo")

import numpy as np
import ml_dtypes
from contextlib import ExitStack

import concourse.bass as bass
import concourse.tile as tile
from concourse import bacc, mybir
from concourse.bass_utils import run_bass_kernel_spmd
from concourse.masks import make_identity

P = 128
NCORES = 8
NS, NG, E = 4096, 20000, 131072
DIN, H, C1, C3 = 256, 4, 64, 128
HC1, HC3 = H * C1, H * C3          # 256, 512
NGP = 20480                        # genes padded to 160 tiles
NST, NGT = NS // P, NGP // P       # 32, 160
S_PER_CORE, G_PER_CORE = NST // NCORES, NGT // NCORES   # 4, 20
SROWS, GROWS = S_PER_CORE * P, G_PER_CORE * P           # 512, 2560

F32 = mybir.dt.float32
BF16 = mybir.dt.bfloat16
I32 = mybir.dt.int32
I16 = mybir.dt.int16
AF = mybir.ActivationFunctionType
OP = mybir.AluOpType
AX = mybir.AxisListType
BF = ml_dtypes.bfloat16

PAD_LOC = 200.0   # dst-local sentinel for padded edges (never equals 0..127)
GMAX = 8          # chunks per gather/DVE batch group
import os
SIM_SAFE = os.environ.get("BIRGAT_SIM_SAFE", "0") == "1"  # CoreSim lacks Prelu


# ---------------------------------------------------------------- host plan

def _deal_tiles(dst, n_tiles, per_core):
    tcnt = np.bincount(dst // P, minlength=n_tiles)
    chunks = np.maximum((tcnt + P - 1) // P, 1)
    order = np.argsort(-chunks, kind="stable")
    assign = order.reshape(per_core, NCORES)
    sched = chunks[assign].max(axis=1)
    return assign, sched.astype(int), tcnt


def _edge_arrays(src, dst, assign, sched, src_row_map, core):
    """Per-core edge chunk arrays for one relation.
    Returns src_rows [P, NCH] i32, dstloc [P, NCH] bf16."""
    nch = int(sched.sum())
    src_rows = np.zeros((nch, P), np.int32)
    dstloc = np.full((nch, P), PAD_LOC, np.float32)
    tile_of = dst // P
    ci = 0
    for slot in range(len(sched)):
        t = assign[slot, core]
        e = np.nonzero(tile_of == t)[0]
        n = len(e)
        want = sched[slot] * P
        s = np.zeros(want, np.int32)
        dl = np.full(want, PAD_LOC, np.float32)
        s[:n] = src_row_map[src[e]]
        dl[:n] = (dst[e] % P).astype(np.float32)
        src_rows[ci:ci + sched[slot]] = s.reshape(-1, P)
        dstloc[ci:ci + sched[slot]] = dl.reshape(-1, P)
        ci += sched[slot]
    idx16 = np.tile(src_rows.reshape(-1, 16).T.astype(np.int16), (8, 1))
    return idx16.copy(), dstloc.T.copy()


def _bcast(v, dt=np.float32, p=P):
    return np.broadcast_to(np.asarray(v, np.float32).reshape(1, -1),
                           (p, len(np.asarray(v).reshape(-1)))).astype(dt)


def _wtile(w, kdim, n):
    """Host-side [128, kdim//128, n] bf16 rhs layout for matmul weights."""
    a = np.asarray(w, np.float32).reshape(kdim // P, P, n)
    return a.transpose(1, 0, 2).astype(BF).copy()


def _xT(x, rows):
    """Host-side [128, DIN//128, rows] bf16 transposed-feature staging."""
    a = np.zeros((DIN, rows), np.float32)
    a[:, :x.shape[0]] = np.asarray(x, np.float32).T
    return a.reshape(DIN // P, P, rows).transpose(1, 0, 2).astype(BF).copy()


def _plan(inputs):
    sg_src = np.asarray(inputs["sg_src"]); sg_dst = np.asarray(inputs["sg_dst"])
    gs_src = np.asarray(inputs["gs_src"]); gs_dst = np.asarray(inputs["gs_dst"])

    g_assign, g_sched, _ = _deal_tiles(sg_dst, NGT, G_PER_CORE)
    s_assign, s_sched, _ = _deal_tiles(gs_dst, NST, S_PER_CORE)

    g_owner = np.zeros(NGT, np.int32); g_slot = np.zeros(NGT, np.int32)
    for slot in range(G_PER_CORE):
        for c in range(NCORES):
            g_owner[g_assign[slot, c]] = c
            g_slot[g_assign[slot, c]] = slot
    s_owner = np.zeros(NST, np.int32); s_slot = np.zeros(NST, np.int32)
    for slot in range(S_PER_CORE):
        for c in range(NCORES):
            s_owner[s_assign[slot, c]] = c
            s_slot[s_assign[slot, c]] = slot

    sid = np.arange(NS)
    srow_tbl = s_owner[sid // P] * SROWS + s_slot[sid // P] * P + sid % P
    gid = np.arange(NG)
    grow_tbl = g_owner[gid // P] * GROWS + g_slot[gid // P] * P + gid % P

    plan = {
        "g_assign": g_assign, "g_sched": g_sched,
        "s_assign": s_assign, "s_sched": s_sched,
    }

    x_sample = np.asarray(inputs["x_sample"], np.float32)
    x_gene = np.asarray(inputs["x_gene"], np.float32)

    in_maps = []
    for c in range(NCORES):
        s_tiles = s_assign[:, c]
        xs_own = x_sample.reshape(NST, P, DIN)[s_tiles].reshape(SROWS, DIN)
        g_tiles = g_assign[:, c]
        xg_own = np.zeros((GROWS, DIN), np.float32)
        for i, t in enumerate(g_tiles):
            lo = t * P
            if lo < NG:
                n = min(P, NG - lo)
                xg_own[i * P:i * P + n] = x_gene[lo:lo + n]

        sgS, sgL = _edge_arrays(sg_src, sg_dst, g_assign, g_sched, srow_tbl, c)
        gsS, gsL = _edge_arrays(gs_src, gs_dst, s_assign, s_sched, grow_tbl, c)

        m = {
            "xsT_own": _xT(xs_own, SROWS),
            "xgT_own": _xT(xg_own, GROWS),
            "Wl1_sg": _wtile(inputs["Wl1_sg"], DIN, HC1),
            "Wr1_sg": _wtile(inputs["Wr1_sg"], DIN, HC1),
            "Wl1_gs": _wtile(inputs["Wl1_gs"], DIN, HC1),
            "Wr1_gs": _wtile(inputs["Wr1_gs"], DIN, HC1),
            "Wl3": _wtile(inputs["Wl3_gs"], HC1, HC3),
            "Wr3": _wtile(inputs["Wr3_gs"], HC1, HC3),
            "sl1_W": _wtile(inputs["sl1_W"], DIN, C1),
            "sl3_W": _wtile(inputs["sl3_W"], HC1, C3),
            "att1_sg_b": _bcast(np.asarray(inputs["att1_sg"]).reshape(-1), BF),
            "att1_gs_b": _bcast(np.asarray(inputs["att1_gs"]).reshape(-1), BF),
            "att3_b": _bcast(np.asarray(inputs["att3_gs"]).reshape(-1), BF),
            "bl1_sg_b": _bcast(inputs["bl1_sg"]),
            "br1_sg_b": _bcast(inputs["br1_sg"]),
            "bl1_gs_b": _bcast(inputs["bl1_gs"]),
            "br1_gs_b": _bcast(inputs["br1_gs"]),
            "bias1_sg_b": _bcast(inputs["bias1_sg"]),
            "bias1_gs_b": _bcast(inputs["bias1_gs"]),
            "bl3_b": _bcast(inputs["bl3_gs"]),
            "br3_b": _bcast(inputs["br3_gs"]),
            "bias3_b": _bcast(inputs["bias3_gs"]),
            "sl1_b_b": _bcast(inputs["sl1_b"]),
            "sl3_b_b": _bcast(inputs["sl3_b"]),
            "sg_idx": sgS, "sg_dstl": sgL,
            "gs_idx": gsS, "gs_dstl": gsL,
        }
        in_maps.append(m)
    return plan, in_maps


# ------------------------------------------------------------- device build

def _groups(nch):
    out = []
    ci = 0
    while ci < nch:
        g = min(GMAX, nch - ci)
        out.append((ci, g))
        ci += g
    return out


def _ap(base, offset_cols, shape_strides):
    """AP over base tile's tensor: shape_strides = [[stride, n], ...] free."""
    a = base[:]
    return bass.AP(a.tensor, a.offset + offset_cols,
                   [[a.ap[0][0], P]] + shape_strides)


def _build(g_sched, s_sched):
    nsg = int(g_sched.sum())
    ngs = int(s_sched.sum())
    nc = bacc.Bacc("TRN2", target_bir_lowering=False, debug=False,
                   num_devices=NCORES)

    ei = lambda name, shape, dt=F32: nc.dram_tensor(name, shape, dt,
                                                    kind="ExternalInput")
    xsT_own = ei("xsT_own", [P, DIN // P, SROWS], BF16)
    xgT_own = ei("xgT_own", [P, DIN // P, GROWS], BF16)
    Wl1_sg = ei("Wl1_sg", [P, 2, HC1], BF16); Wr1_sg = ei("Wr1_sg", [P, 2, HC1], BF16)
    Wl1_gs = ei("Wl1_gs", [P, 2, HC1], BF16); Wr1_gs = ei("Wr1_gs", [P, 2, HC1], BF16)
    Wl3 = ei("Wl3", [P, 2, HC3], BF16); Wr3 = ei("Wr3", [P, 2, HC3], BF16)
    sl1_W = ei("sl1_W", [P, 2, C1], BF16); sl3_W = ei("sl3_W", [P, 2, C3], BF16)
    att1_sg_b = ei("att1_sg_b", [P, HC1], BF16)
    att1_gs_b = ei("att1_gs_b", [P, HC1], BF16)
    att3_b = ei("att3_b", [P, HC3], BF16)
    bl1_sg_b = ei("bl1_sg_b", [P, HC1]); br1_sg_b = ei("br1_sg_b", [P, HC1])
    bl1_gs_b = ei("bl1_gs_b", [P, HC1]); br1_gs_b = ei("br1_gs_b", [P, HC1])
    bias1_sg_b = ei("bias1_sg_b", [P, HC1]); bias1_gs_b = ei("bias1_gs_b", [P, HC1])
    bl3_b = ei("bl3_b", [P, HC3]); br3_b = ei("br3_b", [P, HC3])
    bias3_b = ei("bias3_b", [P, C3])
    sl1_b_b = ei("sl1_b_b", [P, C1]); sl3_b_b = ei("sl3_b_b", [P, C3])
    sg_idx = ei("sg_idx", [P, nsg * 8], I16)
    sg_dstl = ei("sg_dstl", [P, nsg])
    gs_idx = ei("gs_idx", [P, ngs * 8], I16)
    gs_dstl = ei("gs_dstl", [P, ngs])

    out_own = nc.dram_tensor("out_own", [SROWS, C3], F32, kind="ExternalOutput")

    # DRAM scratch (tables bf16)
    agin_s = nc.dram_tensor("agin_s", [SROWS, HC1], BF16)
    agin_g = nc.dram_tensor("agin_g", [GROWS, HC1], BF16)
    agin_3 = nc.dram_tensor("agin_3", [GROWS, HC3], BF16)
    tbl_s = nc.dram_tensor("tbl_s", [NS, HC1], BF16, addr_space="Shared")
    tbl_g = nc.dram_tensor("tbl_g", [NGP, HC1], BF16, addr_space="Shared")
    tbl_3 = nc.dram_tensor("tbl_3", [NGP, HC3], BF16, addr_space="Shared")
    xr1_sg = nc.dram_tensor("xr1_sg", [GROWS, HC1], BF16)
    xr1_gs = nc.dram_tensor("xr1_gs", [SROWS, HC1], BF16)
    xr3 = nc.dram_tensor("xr3", [SROWS, HC3], BF16)

    RG = [list(range(NCORES))]

    with tile.TileContext(nc) as tc, ExitStack() as ctx:
        res = ctx.enter_context(tc.tile_pool(name="res", bufs=1))
        se = ctx.enter_context(tc.tile_pool(name="se", bufs=3))
        sx1 = ctx.enter_context(tc.tile_pool(name="sx1", bufs=3))
        sx3 = ctx.enter_context(tc.tile_pool(name="sx3", bufs=2))
        ev = ctx.enter_context(tc.tile_pool(name="ev", bufs=2))
        psz = ctx.enter_context(tc.tile_pool(name="psz", bufs=2, space="PSUM"))
        psoh = ctx.enter_context(tc.tile_pool(name="psoh", bufs=1, space="PSUM"))
        pse = ctx.enter_context(tc.tile_pool(name="pse", bufs=2, space="PSUM"))
        ps1 = ctx.enter_context(tc.tile_pool(name="ps1", bufs=2, space="PSUM"))
        psd = ctx.enter_context(tc.tile_pool(name="psd", bufs=1, space="PSUM"))

        ident = res.tile([P, P], F32)
        make_identity(nc, ident[:])
        ident_bf = res.tile([P, P], BF16)
        nc.scalar.copy(ident_bf[:], ident[:])
        iota_bf = res.tile([P, P], BF16)
        nc.gpsimd.iota(iota_bf[:], pattern=[[1, P]], base=0,
                       channel_multiplier=0,
                       allow_small_or_imprecise_dtypes=True)
        ones_bf = res.tile([P, 1], BF16)
        nc.vector.memset(ones_bf[:], 1.0)

        def rload(name, dram, shape, dt=F32):
            t = res.tile(shape, dt, tag=name)
            nc.sync.dma_start(t[:], dram[:])
            return t

        att1_sg_t = rload("a1s", att1_sg_b, [P, HC1], BF16)
        att1_gs_t = rload("a1g", att1_gs_b, [P, HC1], BF16)
        att3_t = rload("a3", att3_b, [P, HC3], BF16)
        bias1_sg_t = rload("b1s", bias1_sg_b, [P, HC1])
        bias1_gs_t = rload("b1g", bias1_gs_b, [P, HC1])
        bias3_t = rload("b3", bias3_b, [P, C3])
        sg_idx_t = rload("sgs", sg_idx, [P, nsg * 8], I16)
        sg_dstl_t = rload("sgl", sg_dstl, [P, nsg])
        gs_idx_t = rload("gss", gs_idx, [P, ngs * 8], I16)
        gs_dstl_t = rload("gsl", gs_dstl, [P, ngs])
        xsT = rload("xsT", xsT_own, [P, DIN // P, SROWS], BF16)
        xgT = rload("xgT", xgT_own, [P, DIN // P, GROWS], BF16)

        Wl1_sg_t = rload("Wl1_sg", Wl1_sg, [P, 2, HC1], BF16)
        Wr1_sg_t = rload("Wr1_sg", Wr1_sg, [P, 2, HC1], BF16)
        Wl1_gs_t = rload("Wl1_gs", Wl1_gs, [P, 2, HC1], BF16)
        Wr1_gs_t = rload("Wr1_gs", Wr1_gs, [P, 2, HC1], BF16)
        Wl3_t = rload("Wl3", Wl3, [P, 2, HC3], BF16)
        Wr3_t = rload("Wr3", Wr3, [P, 2, HC3], BF16)
        sl1_W_t = rload("sl1_W", sl1_W, [P, 2, C1], BF16)
        sl3_W_t = rload("sl3_W", sl3_W, [P, 2, C3], BF16)
        bl1_sg_t = rload("bl1s", bl1_sg_b, [P, HC1])
        br1_sg_t = rload("br1s", br1_sg_b, [P, HC1])
        bl1_gs_t = rload("bl1g", bl1_gs_b, [P, HC1])
        br1_gs_t = rload("br1g", br1_gs_b, [P, HC1])
        bl3_t = rload("bl3", bl3_b, [P, HC3])
        br3_t = rload("br3", br3_b, [P, HC3])
        sl1_b_t = rload("sl1b", sl1_b_b, [P, C1])
        sl3_b_t = rload("sl3b", sl3_b_b, [P, C3])

        sl1_sb = res.tile([P, S_PER_CORE * C1], F32)
        sl3_sb = res.tile([P, S_PER_CORE * C3], F32)

        def dense_T(lhsT, w_sb, n, bias_t, dst_ap=None, sbuf_dst=None,
                    rdt=BF16, tag="dout"):
            """out = lhsT.T @ W + bias; lhsT = list of [128,128] bf16 APs."""
            pt = ps1.tile([P, HC3], F32, space="PSUM", tag="aux")
            for k in range(len(lhsT)):
                nc.tensor.matmul(pt[:, :n], lhsT=lhsT[k], rhs=w_sb[:, k, :n],
                                 start=(k == 0), stop=(k == len(lhsT) - 1))
            o = sbuf_dst
            if o is None:
                ot = se.tile([P, n], rdt, tag=tag)
                o = ot[:]
            nc.vector.tensor_tensor(out=o, in0=pt[:, :n], in1=bias_t[:, :n],
                                    op=OP.add)
            if dst_ap is not None:
                nc.sync.dma_start(dst_ap, ot[:])
            return o

        # ---- phase A: node tables (no on-chip transposes; xT staged)
        for i in range(S_PER_CORE):
            lhsT = [xsT[:, k, i * P:(i + 1) * P] for k in range(DIN // P)]
            dense_T(lhsT, Wl1_sg_t, HC1, bl1_sg_t,
                    dst_ap=agin_s[i * P:(i + 1) * P, :])
            dense_T(lhsT, Wr1_gs_t, HC1, br1_gs_t,
                    dst_ap=xr1_gs[i * P:(i + 1) * P, :])
            dense_T(lhsT, sl1_W_t, C1, sl1_b_t,
                    sbuf_dst=sl1_sb[:, i * C1:(i + 1) * C1])
        nc.gpsimd.collective_compute("AllGather", OP.bypass, replica_groups=RG,
                                     ins=[agin_s[:]], outs=[tbl_s[:]])

        for j in range(G_PER_CORE):
            lhsT = [xgT[:, k, j * P:(j + 1) * P] for k in range(DIN // P)]
            dense_T(lhsT, Wl1_gs_t, HC1, bl1_gs_t,
                    dst_ap=agin_g[j * P:(j + 1) * P, :])
            dense_T(lhsT, Wr1_sg_t, HC1, br1_sg_t,
                    dst_ap=xr1_sg[j * P:(j + 1) * P, :])
        nc.gpsimd.collective_compute("AllGather", OP.bypass, replica_groups=RG,
                                     ins=[agin_g[:]], outs=[tbl_g[:]])

        # ---- edge slot processing
        def edge_slot(ci0, nch, idx_t, dstl_t, tbl, xr_sb, att_t, w, pm, pden):
            """Process one dst slot's nch chunks; accumulate into pm (+pden
            for layer3 mode, which is signalled by pden is not None)."""
            l3 = pden is not None
            sx = sx3 if l3 else sx1
            nfold = 4 if w == HC3 else 3   # per-head 128->8 or 64->8
            ch = w // H
            for gi, (goff, G) in enumerate(_groups(nch)):
                ci = ci0 + goff
                first0 = goff == 0
                # batched gather [128, G, w]
                xl = sx.tile([P, GMAX, w], BF16, tag=f"xl{w}")
                nc.gpsimd.dma_gather(
                    out_ap=_ap(xl, 0, [[w, G], [1, w]]), in_ap=tbl[:],
                    idxs_ap=idx_t[:, ci * 8:(ci + G) * 8],
                    num_idxs=G * P, num_idxs_reg=G * P, elem_size=w)
                # one-hots (bf16, 4x TSPtr) + PE transposes -> ohT bf16
                onehot = se.tile([P, GMAX, P], BF16, tag="onehot")
                ohT = se.tile([P, GMAX, P], BF16, tag="ohT")
                for g in range(G):
                    nc.vector.tensor_scalar(
                        out=onehot[:, g, :], in0=iota_bf[:],
                        scalar1=dstl_t[:, ci + g:ci + g + 1], scalar2=None,
                        op0=OP.is_equal)
                pt = psoh.tile([P, GMAX, P], BF16, space="PSUM", tag="ohTp")
                for g in range(G):
                    nc.tensor.transpose(out=pt[:, g, :],
                                        in_=onehot[:, g, :],
                                        identity=ident_bf[:])
                nc.scalar.activation(
                    _ap(ohT, 0, [[P, G], [1, P]]),
                    _ap(pt, 0, [[P, G], [1, P]]), AF.Copy)
                # z = ident@xl + ohT@xr (PSUM fp32), Prelu -> gt bf16
                # (gt doubles as the msgs tile; ea cols duplicated in pairs
                # so the xl*ea broadcast TT keeps a packed innermost dim=2
                # and hits the DVE 2x mode)
                wh = w + 2 * H
                gt = sx.tile([P, GMAX, wh], BF16, tag=f"g{w}")
                gm = sx.tile([P, GMAX, w], BF16, tag=f"gm{w}")
                zb = 1 if l3 else 2   # chunks per PSUM z tile (2KB bank)
                for z0 in range(0, G, zb):
                    zn = min(zb, G - z0)
                    zp = psz.tile([P, HC3], F32, space="PSUM", tag="z")
                    for g in range(z0, z0 + zn):
                        zv = _ap(zp, (g - z0) * w, [[1, w]])
                        nc.tensor.matmul(zv, lhsT=ident_bf[:],
                                         rhs=xl[:, g, :], start=True,
                                         stop=False)
                        nc.tensor.matmul(zv, lhsT=ohT[:, g, :],
                                         rhs=xr_sb[:, :w], start=False,
                                         stop=True)
                    if SIM_SAFE:
                        nc.vector.scalar_tensor_tensor(
                            out=_ap(gt, z0 * wh, [[wh, zn], [1, w]]),
                            in0=_ap(zp, 0, [[w, zn], [1, w]]), scalar=0.2,
                            in1=_ap(zp, 0, [[w, zn], [1, w]]),
                            op0=OP.mult, op1=OP.max)
                    else:
                        nc.scalar.activation(
                            _ap(gt, z0 * wh, [[wh, zn], [1, w]]),
                            _ap(zp, 0, [[w, zn], [1, w]]), AF.Prelu, alpha=0.2)
                    # alpha stage 1: gm = g*att (bf16 2x), per z-pair
                    nc.vector.tensor_tensor(
                        out=_ap(gm, z0 * w, [[w, zn], [1, w]]),
                        in0=_ap(gt, z0 * wh, [[wh, zn], [1, w]]),
                        in1=_ap(att_t, 0, [[0, zn], [1, w]]), op=OP.mult)
                src_t, src_w, src_off = gm, w, 0
                for f in range(nfold):
                    hw_ = src_w // H // 2       # half of per-head width
                    ft = sx.tile([P, GMAX * H * hw_], BF16, tag=f"f{w}_{f}")
                    nc.vector.tensor_tensor(
                        out=_ap(ft, 0, [[hw_, G * H], [1, hw_]]),
                        in0=_ap(src_t, src_off, [[2 * hw_, G * H], [1, hw_]]),
                        in1=_ap(src_t, src_off + hw_,
                                [[2 * hw_, G * H], [1, hw_]]),
                        op=OP.add)
                    src_t, src_w, src_off = ft, src_w // 2, 0
                alpha = se.tile([P, GMAX * H], F32, tag="alpha")
                nc.vector.tensor_reduce(
                    out=_ap(alpha, 0, [[1, G * H]]),
                    in_=_ap(src_t, 0, [[8, G * H], [1, 8]]),
                    axis=AX.X, op=OP.add)
                nc.scalar.activation(
                    _ap(gt, w, [[wh, G], [2, H]]),
                    _ap(alpha, 0, [[H, G], [1, H]]), AF.Exp)
                nc.vector.tensor_scalar(
                    out=_ap(gt, w + 1, [[wh, G], [2, H]]),
                    in0=_ap(gt, w, [[wh, G], [2, H]]),
                    scalar1=1.0, scalar2=None, op0=OP.mult)
                nc.vector.tensor_tensor(
                    out=_ap(gt, 0, [[wh, G], [ch, H], [2, ch // 2], [1, 2]]),
                    in0=_ap(xl, 0, [[w, G], [ch, H], [2, ch // 2], [1, 2]]),
                    in1=_ap(gt, w, [[wh, G], [2, H], [0, ch // 2], [1, 2]]),
                    op=OP.mult)
                for g in range(G):
                    st = first0 and g == 0
                    sp = goff + g == nch - 1
                    if l3:
                        nc.tensor.matmul(pm[:, :w], lhsT=onehot[:, g, :],
                                         rhs=gt[:, g, :w], start=st, stop=sp)
                        nc.tensor.matmul(pden[:, :H], lhsT=onehot[:, g, :],
                                         rhs=_ap(gt, g * wh + w, [[2, H]]),
                                         start=st, stop=sp)
                    else:
                        nc.tensor.matmul(pm[:, :wh], lhsT=onehot[:, g, :],
                                         rhs=gt[:, g, :], start=st, stop=sp)

        def norm_heads(psum_ap, den_ap, w, tag):
            ch = w // H
            den = ev.tile([P, H], F32, tag="den")
            nc.vector.tensor_scalar(out=den[:], in0=den_ap, scalar1=1e-16,
                                    scalar2=None, op0=OP.add)
            rden = ev.tile([P, H], F32, tag="rden")
            nc.vector.reciprocal(rden[:], den[:])
            y = ev.tile([P, w], F32, tag=tag)
            nc.vector.tensor_tensor(
                out=_ap(y, 0, [[ch, H], [1, ch]]),
                in0=psum_ap,
                in1=_ap(rden, 0, [[1, H], [0, ch]]),
                op=OP.mult)
            return y

        def _elu(out_ap, y_ap, w):
            m = ev.tile([P, w], F32, tag="elu_m")
            nc.vector.tensor_scalar(out=m[:], in0=y_ap, scalar1=0.0,
                                    scalar2=None, op0=OP.min)
            e = ev.tile([P, w], F32, tag="elu_e")
            nc.scalar.activation(e[:], m[:], AF.Exp)
            r = ev.tile([P, w], F32, tag="elu_r")
            nc.scalar.activation(r[:], y_ap, AF.Relu)
            nc.vector.scalar_tensor_tensor(out=out_ap, in0=r[:], scalar=-1.0,
                                           in1=e[:], op0=OP.add, op1=OP.add)

        def x1_transposed(x1_ap, tag):
            """fp32 x1 [128, HC1] -> bf16, PE-transpose -> 2 bf16 lhsT."""
            xb = ev.tile([P, HC1], BF16, tag=tag + "b")
            nc.scalar.activation(xb[:], x1_ap, AF.Copy)
            pt = psoh.tile([P, GMAX, P], BF16, space="PSUM", tag="ohTp")
            for k in range(2):
                nc.tensor.transpose(out=pt[:, k, :], in_=xb[:, k * P:(k + 1) * P],
                                    identity=ident_bf[:])
            xT2 = ev.tile([P, 2, P], BF16, tag=tag + "T")
            nc.scalar.activation(_ap(xT2, 0, [[P, 2], [1, P]]),
                                 _ap(pt, 0, [[P, 2], [1, P]]), AF.Copy)
            return [xT2[:, 0, :], xT2[:, 1, :]]

        # ---- phase B: sg edges -> x1_gene -> xl3 rows, AG2
        ci = 0
        for slot in range(G_PER_CORE):
            xr_sb = se.tile([P, HC1], BF16, tag="xr1")
            nc.sync.dma_start(xr_sb[:], xr1_sg[slot * P:(slot + 1) * P, :])
            pm = pse.tile([P, HC3], F32, space="PSUM", tag="pm")
            edge_slot(ci, int(g_sched[slot]), sg_idx_t, sg_dstl_t, tbl_s,
                      xr_sb, att1_sg_t, HC1, pm, None)
            ci += int(g_sched[slot])
            y = norm_heads(pm[:, :HC1], _ap(pm, HC1, [[2, H]]), HC1, "y1g")
            y2 = ev.tile([P, HC1], F32, tag="y2g")
            nc.vector.tensor_tensor(out=y2[:], in0=y[:], in1=bias1_sg_t[:],
                                    op=OP.add)
            x1 = ev.tile([P, HC1], F32, tag="x1g")
            _elu(x1[:], y2[:], HC1)
            lhsT = x1_transposed(x1[:], "x1gT")
            dense_T(lhsT, Wl3_t, HC3, bl3_t,
                    dst_ap=agin_3[slot * P:(slot + 1) * P, :])
        nc.gpsimd.collective_compute("AllGather", OP.bypass, replica_groups=RG,
                                     ins=[agin_3[:]], outs=[tbl_3[:]])

        # ---- phase C: gs edges -> x1_sample -> xr3/sl3 rows
        ci = 0
        for slot in range(S_PER_CORE):
            xr_sb = se.tile([P, HC1], BF16, tag="xr1")
            nc.sync.dma_start(xr_sb[:], xr1_gs[slot * P:(slot + 1) * P, :])
            pm = pse.tile([P, HC3], F32, space="PSUM", tag="pm")
            edge_slot(ci, int(s_sched[slot]), gs_idx_t, gs_dstl_t, tbl_g,
                      xr_sb, att1_gs_t, HC1, pm, None)
            ci += int(s_sched[slot])
            y = norm_heads(pm[:, :HC1], _ap(pm, HC1, [[2, H]]), HC1, "y1s")
            y2 = ev.tile([P, HC1], F32, tag="y2s")
            nc.vector.tensor_tensor(out=y2[:], in0=y[:], in1=bias1_gs_t[:],
                                    op=OP.add)
            y3 = ev.tile([P, HC1], F32, tag="y3s")
            nc.vector.tensor_tensor(
                out=_ap(y3, 0, [[C1, H], [1, C1]]),
                in0=_ap(y2, 0, [[C1, H], [1, C1]]),
                in1=_ap(sl1_sb, slot * C1, [[0, H], [1, C1]]),
                op=OP.add)
            x1 = ev.tile([P, HC1], F32, tag="x1s")
            _elu(x1[:], y3[:], HC1)
            lhsT = x1_transposed(x1[:], "x1sT")
            dense_T(lhsT, Wr3_t, HC3, br3_t,
                    dst_ap=xr3[slot * P:(slot + 1) * P, :])
            dense_T(lhsT, sl3_W_t, C3, sl3_b_t,
                    sbuf_dst=sl3_sb[:, slot * C3:(slot + 1) * C3])

        # ---- phase D: gs edges layer 3 -> output
        ci = 0
        for slot in range(S_PER_CORE):
            xr_sb = se.tile([P, HC3], BF16, tag="xr3")
            nc.sync.dma_start(xr_sb[:], xr3[slot * P:(slot + 1) * P, :])
            pm = pse.tile([P, HC3], F32, space="PSUM", tag="pm")
            pden = psd.tile([P, H], F32, space="PSUM", tag="pden")
            edge_slot(ci, int(s_sched[slot]), gs_idx_t, gs_dstl_t, tbl_3,
                      xr_sb, att3_t, HC3, pm, pden)
            ci += int(s_sched[slot])
            den4 = ev.tile([P, H], F32, tag="den")
            nc.vector.tensor_scalar(out=den4[:], in0=pden[:], scalar1=4.0,
                                    scalar2=4e-16, op0=OP.mult, op1=OP.add)
            rden = ev.tile([P, H], F32, tag="rden")
            nc.vector.reciprocal(rden[:], den4[:])
            base = ev.tile([P, C3], F32, tag="based")
            nc.vector.tensor_tensor(out=base[:],
                                    in0=sl3_sb[:, slot * C3:(slot + 1) * C3],
                                    in1=bias3_t[:], op=OP.add)
            accs = [base]
            for h in range(H):
                a = ev.tile([P, C3], F32, tag=f"acc{h}")
                nc.vector.scalar_tensor_tensor(
                    out=a[:], in0=pm[:, h * C3:(h + 1) * C3],
                    scalar=rden[:, h:h + 1], in1=accs[-1][:],
                    op0=OP.mult, op1=OP.add)
                accs.append(a)
            o = ev.tile([P, C3], F32, tag="outt")
            _elu(o[:], accs[-1][:], C3)
            nc.sync.dma_start(out_own[slot * P:(slot + 1) * P, :], o[:])

    nc.compile()
    return nc


# ------------------------------------------------------------------ driver

_CACHE = {}


def kernel(**inputs):
    plan, in_maps = _plan(inputs)
    key = (tuple(plan["g_sched"]), tuple(plan["s_sched"]))
    if key not in _CACHE:
        _CACHE[key] = _build(plan["g_sched"], plan["s_sched"])
    nc = _CACHE[key]
    r = run_bass_kernel_spmd(nc, in_maps, core_ids=list(range(NCORES)))
    out = np.zeros((NS, C3), np.float32)
    s_assign = plan["s_assign"]
    for c in range(NCORES):
        oc = r.results[c]["out_own"]
        for slot in range(S_PER_CORE):
            t = s_assign[slot, c]
            out[t * P:(t + 1) * P] = oc[slot * P:(slot + 1) * P]
    return out
